# revision 14
# baseline (speedup 1.0000x reference)
"""nn_Attention Trainium2 Bass kernel (v2 — interleaved pipeline).

Full attention forward: x->(q,k,v) with l2-normalized weights, per-head-dim
l2 norm + learned qk scale, interleaved RoPE, causal SDPA, output projection
with column-l2-normalized wo.

Sharding: TP=4 over heads (8 heads/core) x DP=2 over batch across 8 cores.
Each core computes a partial [2048, 2048] output for its batch; host sums
the 4 TP partials per batch.

v2 changes vs v1:
- single interleaved loop per 512-row block: proj -> attention -> yproj,
  so DVE rope work, Act exp work and PE matmuls overlap across phases.
- q/k transposes via DMA xbar (dma_start_transpose) instead of PE
  transposes + DVE copies.
- causal mask as a single 128x128 triangle multiply on the Pool engine.
- lg/pv matmuls trimmed to the live columns on diagonal blocks.
- softmax denominators: v's 65th ones-column -> psum row 64 -> stashed ->
  gathered by DMA -> PE-transposed to si-partition layout -> one cheap
  [128,32] reciprocal -> transposed back -> rank-8 indicator matmul
  broadcast (replaces 3.3us-per-call wide DVE reciprocals).
- x streamed per 512-column block (2-deep) instead of fully resident.
- yproj results DMA'd directly from PSUM to DRAM.
"""
import sys
import os
import math
from contextlib import ExitStack

sys.path.insert(0, "/opt/trn_rl_repo")

import numpy as np
import ml_dtypes

BF16 = ml_dtypes.bfloat16

B, S, DIM = 2, 2048, 2048
HEADS, DH = 32, 64
THETA = 10000.0
NCORES = 8
TP = 4             # head-parallel ways
HPC = HEADS // TP  # heads per core = 8
E = HPC * DH       # per-core qkv width = 512
ET = E // 128      # e-tiles per core = 4
DT = DIM // 128    # contraction d-tiles = 16
SB = S // 512      # 512-wide seq blocks = 4
SS = S // 128      # 128-wide seq blocks = 16

_CACHE = {}


def _l2n(w, axis):
    n = np.sqrt((w.astype(np.float64) ** 2).sum(axis=axis, keepdims=True))
    n = np.maximum(n, 1e-12)
    return (w / n).astype(np.float32)


def _build_program():
    import concourse.bass as bass
    from concourse import bacc
    import concourse.mybir as mybir
    import concourse.tile as tile
    from concourse.masks import make_identity

    f32 = mybir.dt.float32
    bf16 = mybir.dt.bfloat16
    AF = mybir.ActivationFunctionType
    AX = mybir.AxisListType
    OP = mybir.AluOpType

    nc = bacc.Bacc("TRN2", target_bir_lowering=False)

    xT = nc.dram_tensor("xT", [DIM, S], bf16, kind="ExternalInput")
    wqT = nc.dram_tensor("wqT", [DIM, E], bf16, kind="ExternalInput")
    wkT = nc.dram_tensor("wkT", [DIM, E], bf16, kind="ExternalInput")
    wvT = nc.dram_tensor("wvT", [DIM, E], bf16, kind="ExternalInput")
    woT = nc.dram_tensor("woT", [E, DIM], bf16, kind="ExternalInput")
    cosd = nc.dram_tensor("cosd", [128, SS * DH], bf16, kind="ExternalInput")
    sind = nc.dram_tensor("sind", [128, SS * DH], bf16, kind="ExternalInput")
    trid = nc.dram_tensor("trid", [128, 128], bf16, kind="ExternalInput")
    ind8d = nc.dram_tensor("ind8d", [8, 512], bf16, kind="ExternalInput")
    Y = nc.dram_tensor("Y", [S, DIM], f32, kind="ExternalOutput")

    with tile.TileContext(nc) as tc, ExitStack() as ctx:
        const = ctx.enter_context(tc.tile_pool(name="const", bufs=1))
        wpool = ctx.enter_context(tc.tile_pool(name="wpool", bufs=4))
        xpool = ctx.enter_context(tc.tile_pool(name="xpool", bufs=2))
        qkv = ctx.enter_context(tc.tile_pool(name="qkv", bufs=1))
        work = ctx.enter_context(tc.tile_pool(name="work", bufs=1))
        expool = ctx.enter_context(tc.tile_pool(name="expool", bufs=3))
        psA = ctx.enter_context(
            tc.tile_pool(name="psA", bufs=4, space="PSUM"))
        psL = ctx.enter_context(
            tc.tile_pool(name="psL", bufs=2, space="PSUM"))

        # --- weights (wq first, quartered, so proj can start early) ---
        wq_sb = [wpool.tile([128, 4, E], bf16, tag=f"wq{j}", bufs=1, name=f"wq{j}")
                 for j in range(4)]
        wk_sb = wpool.tile([128, DT, E], bf16, tag="wk", bufs=1)
        wv_sb = wpool.tile([128, DT, E], bf16, tag="wv", bufs=1)
        wo_sb = wpool.tile([128, ET, DIM], bf16, tag="wo", bufs=1)
        wqr = wqT.rearrange("(t p) e -> p t e", p=128)

        xtiles = {}

        def load_x(st):
            ts = [xpool.tile([128, 4, 512], bf16, tag=f"x{j}", bufs=2,
                             name=f"xst{st}_{j}") for j in range(4)]
            src = xT[:, st * 512:(st + 1) * 512].rearrange(
                "(t p) s -> p t s", p=128)
            for j in range(4):
                nc.sync.dma_start(ts[j], src[:, j * 4:(j + 1) * 4, :])
            return ts

        # interleave wq quarters with x quarters so dt=0..3 can start early
        x0src = xT[:, 0:512].rearrange("(t p) s -> p t s", p=128)
        x0 = [xpool.tile([128, 4, 512], bf16, tag=f"x{j}", bufs=2,
                         name=f"xst0_{j}") for j in range(4)]
        for j in range(4):
            nc.sync.dma_start(wq_sb[j], wqr[:, j * 4:(j + 1) * 4, :])
            nc.sync.dma_start(x0[j], x0src[:, j * 4:(j + 1) * 4, :])
        xtiles[0] = x0
        nc.sync.dma_start(wk_sb, wkT.rearrange("(t p) e -> p t e", p=128))
        nc.sync.dma_start(wv_sb, wvT.rearrange("(t p) e -> p t e", p=128))

        # --- constants ---
        cos_sb = const.tile([128, SS, DH], bf16)
        sin_sb = const.tile([128, SS, DH], bf16)
        nc.sync.dma_start(cos_sb, cosd.rearrange("p (b d) -> p b d", d=DH))
        nc.sync.dma_start(sin_sb, sind.rearrange("p (b d) -> p b d", d=DH))
        tri = const.tile([128, 128], bf16)
        nc.sync.dma_start(tri, trid[:, :])
        ind8 = const.tile([8, 512], bf16)
        nc.sync.dma_start(ind8, ind8d[:, :])
        nc.sync.dma_start(wo_sb, woT.rearrange("(t p) e -> p t e", p=128))
        identf = const.tile([128, 128], f32)
        make_identity(nc, identf)
        ident = const.tile([128, 128], bf16)
        make_identity(nc, ident)

        # --- persistent activations ---
        qTall = qkv.tile([128, ET, S], bf16, tag="qT")
        kTall = qkv.tile([128, ET, S], bf16, tag="kT")
        v_sb = qkv.tile([128, SS, HPC, 65], bf16, tag="v")
        stash = qkv.tile([65, HPC, 512], bf16, tag="stash")
        nc.vector.memset(v_sb[:, :, :, 64:65], 1.0)

        def norm_rope(ps, dstT, st, su):
            """psum [si,e] natural -> per-head l2norm, rope, bf16,
            -> DMA-transpose into dstT columns."""
            sblk = st * 4 + su
            sq = work.tile([128, E], bf16, tag="sq", bufs=2)
            nc.scalar.square(sq, ps)
            ssq = work.tile([128, HPC], f32, tag="ssq", bufs=2)
            nc.vector.tensor_reduce(
                ssq, sq.rearrange("p (h d) -> p h d", d=DH),
                axis=AX.X, op=OP.add)
            nc.scalar.activation(ssq, ssq, AF.Ln)
            inv = work.tile([128, HPC], f32, tag="inv", bufs=2)
            nc.scalar.activation(inv, ssq, AF.Exp, scale=-0.5)
            qn = work.tile([128, HPC, DH], bf16, tag="qn", bufs=2)
            nc.vector.tensor_mul(
                qn, ps.rearrange("p (h d) -> p h d", d=DH),
                inv.unsqueeze(2).broadcast_to([128, HPC, DH]))
            cosb = cos_sb[:, sblk:sblk + 1, :].broadcast_to([128, HPC, DH])
            sinb = sin_sb[:, sblk:sblk + 1, :].broadcast_to([128, HPC, DH])
            rot = work.tile([128, HPC, 2, 32], bf16, tag="rot", bufs=2)
            qn4 = qn.rearrange("p h (t u) -> p h t u", u=32)
            nc.gpsimd.tensor_copy(rot[:, :, 0:1, :], qn4[:, :, 1:2, :])
            nc.gpsimd.tensor_copy(rot[:, :, 1:2, :], qn4[:, :, 0:1, :])
            nc.gpsimd.tensor_mul(rot.rearrange("p h t u -> p h (t u)"),
                                 rot.rearrange("p h t u -> p h (t u)"), sinb)
            nc.gpsimd.tensor_mul(qn, qn, cosb)
            qo = work.tile([128, E], bf16, tag="qo", bufs=2)
            nc.vector.tensor_add(
                qo, qn.rearrange("p h d -> p (h d)"),
                rot.rearrange("p h t u -> p (h t u)"))
            nc.sync.dma_start_transpose(
                dstT[:, :, sblk * 128:(sblk + 1) * 128], qo)

        def proj_wave(w_sb, kind, st, xt):
            pss = [psA.tile([128, E], f32, tag="ps", name=f"p{kind}{st}_{su}")
                   for su in range(4)]
            quartered = isinstance(w_sb, list)
            for dt in range(DT):
                if quartered:
                    wslice = w_sb[dt // 4][:, dt % 4, :]
                else:
                    wslice = w_sb[:, dt, :]
                for su in range(4):
                    nc.tensor.matmul(
                        pss[su],
                        xt[dt // 4][:, dt % 4, su * 128:(su + 1) * 128],
                        wslice,
                        start=(dt == 0), stop=(dt == DT - 1))
            for su in range(4):
                if kind == "v":
                    nc.vector.tensor_copy(
                        v_sb[:, st * 4 + su, :, 0:64],
                        pss[su].rearrange("p (h d) -> p h d", d=DH))
                else:
                    norm_rope(pss[su], qTall if kind == "q" else kTall,
                              st, su)

        def proj_all(st):
            xt = xtiles[st]
            proj_wave(wq_sb, "q", st, xt)
            proj_wave(wk_sb, "k", st, xt)
            proj_wave(wv_sb, "v", st, xt)

        def attn_block(i):
            last = 4 * i + 3
            for h in range(HPC):
                et, hp = h // 2, (h % 2) * 64
                pv = psA.tile([128, 512], f32, tag="ps", name=f"pv{i}_{h}")
                npr = 2 * (i + 1)
                lgs = {}

                def emit_lg(p):
                    lg2 = psL.tile([128, 2, 512], f32, tag="lg",
                                   name=f"lg{i}_{h}_{p}")
                    for b in range(2):
                        sjb = 2 * p + b
                        r = sjb - 4 * i
                        c0 = r * 128 if r > 0 else 0
                        nc.tensor.matmul(
                            lg2[:, b, c0:],
                            kTall[hp:hp + 64, et, sjb * 128:(sjb + 1) * 128],
                            qTall[hp:hp + 64, et,
                                  i * 512 + c0:(i + 1) * 512],
                            start=True, stop=True)
                    lgs[p] = lg2

                emit_lg(0)
                if npr > 1:
                    emit_lg(1)
                for p in range(npr):
                    lg2 = lgs.pop(p)
                    ex = expool.tile([128, 2, 512], bf16, tag="ex")
                    if 2 * p - 4 * i >= 0:  # diagonal pair: match lg trim
                        for b in range(2):
                            c0 = max(0, (2 * p + b - 4 * i)) * 128
                            nc.scalar.activation(ex[:, b, c0:],
                                                 lg2[:, b, c0:], AF.Exp)
                    else:
                        nc.scalar.activation(ex, lg2, AF.Exp)
                    if p + 2 < npr:
                        emit_lg(p + 2)
                    for b in range(2):
                        sjb = 2 * p + b
                        r = sjb - 4 * i
                        if r >= 0:
                            nc.gpsimd.tensor_mul(
                                ex[:, b, r * 128:(r + 1) * 128],
                                ex[:, b, r * 128:(r + 1) * 128], tri)
                        c0 = r * 128 if r > 0 else 0
                        nc.tensor.matmul(
                            pv[0:65, c0:],
                            v_sb[:, sjb, h, :],
                            ex[:, b, c0:],
                            start=(sjb == 0), stop=(sjb == last))
                nc.vector.tensor_copy(stash[:, h, :], pv[0:65, :])

        def normalize_gather(i):
            den = work.tile([8, 512], bf16, tag="den", bufs=2)
            nc.scalar.dma_start(den, stash[64:65, :, :])
            return den

        def normalize_apply(i, den):
            """1/den in si-partition layout via PE transposes, then rank-8
            indicator broadcast + per-head mul into qTall."""
            invT = psA.tile([128, 32], bf16, tag="ps")
            for c in range(4):
                nc.tensor.transpose(
                    invT[:, c * 8:(c + 1) * 8],
                    den[:, c * 128:(c + 1) * 128], ident[0:8, 0:8])
            inv_sb = work.tile([128, 32], f32, tag="invsb", bufs=2)
            nc.vector.reciprocal(inv_sb, invT)
            invrow = psA.tile([8, 4, 128], f32, tag="ps")
            for c in range(4):
                nc.tensor.transpose(
                    invrow[:, c, :], inv_sb[:, c * 8:(c + 1) * 8], identf)
            inv_row = work.tile([8, 512], bf16, tag="invrowsb", bufs=2)
            nc.vector.tensor_copy(
                inv_row, invrow.rearrange("p c j -> p (c j)"))
            for h in range(HPC):
                et, hp = h // 2, (h % 2) * 64
                bc = psA.tile([64, 512], f32, tag="ps", name=f"bc{i}_{h}")
                nc.tensor.matmul(bc, ind8[:, h * 64:(h + 1) * 64], inv_row,
                                 start=True, stop=True)
                nc.vector.tensor_mul(
                    qTall[hp:hp + 64, et, i * 512:(i + 1) * 512],
                    stash[0:64, h, :], bc)

        def yproj_block(i):
            for ib in range(4 * i, 4 * i + 4):
                for nd in range(4):
                    ps = psA.tile([128, 512], f32, tag="ps",
                                  name=f"y{ib}_{nd}")
                    for ket in range(ET):
                        nc.tensor.matmul(
                            ps,
                            qTall[:, ket, ib * 128:(ib + 1) * 128],
                            wo_sb[:, ket, nd * 512:(nd + 1) * 512],
                            start=(ket == 0), stop=(ket == ET - 1))
                    ys = work.tile([128, 512], f32, tag="ys", bufs=2)
                    if nd % 2 == 0:
                        nc.vector.tensor_copy(ys, ps)
                    else:
                        nc.scalar.copy(ys, ps)
                    nc.sync.dma_start(
                        Y[ib * 128:(ib + 1) * 128, nd * 512:(nd + 1) * 512],
                        ys)

        proj_all(0)
        for st in range(SB):
            if st + 1 < SB:
                xtiles[st + 1] = load_x(st + 1)
            attn_block(st)
            den = normalize_gather(st)
            if st + 1 < SB:
                proj_wave(wq_sb, "q", st + 1, xtiles[st + 1])
                normalize_apply(st, den)
                proj_wave(wk_sb, "k", st + 1, xtiles[st + 1])
                proj_wave(wv_sb, "v", st + 1, xtiles[st + 1])
            else:
                normalize_apply(st, den)
            yproj_block(st)

    return nc


def _host_prep(x, wq, wk, wv, wo, qk_scale):
    """Returns per-core input dicts."""
    perm = np.concatenate([np.arange(0, DH, 2), np.arange(1, DH, 2)])
    wq_n = _l2n(wq, -1).reshape(HEADS, DH, DIM)[:, perm, :].reshape(HEADS * DH, DIM)
    wk_n = _l2n(wk, -1).reshape(HEADS, DH, DIM)[:, perm, :].reshape(HEADS * DH, DIM)
    wv_n = _l2n(wv, -1)
    wo_n = _l2n(wo, 0)
    sp = qk_scale.astype(np.float64)[perm]

    # rope tables with qk_scale folded in; permuted-block layout
    half = np.arange(0, DH, 2)
    freqs = 1.0 / (THETA ** (half.astype(np.float64) / DH))      # (32,)
    ang = np.arange(S, dtype=np.float64)[:, None] * freqs[None]  # (S, 32)
    cos_h, sin_h = np.cos(ang), np.sin(ang)
    cos_p = np.concatenate([cos_h, cos_h], 1)                    # (S, 64)
    sin_e = np.concatenate([-sin_h, sin_h], 1)
    cos_eff = (cos_p * sp[None, :]).astype(np.float32)
    swap_sp = np.concatenate([sp[32:], sp[:32]])
    sin_eff = (sin_e * swap_sp[None, :]).astype(np.float32)
    # device layout [128, SS*DH]: [p, b*64+c] = tbl[b*128+p, c]
    cosd = np.ascontiguousarray(
        cos_eff.reshape(SS, 128, DH).transpose(1, 0, 2).reshape(128, SS * DH))
    sind = np.ascontiguousarray(
        sin_eff.reshape(SS, 128, DH).transpose(1, 0, 2).reshape(128, SS * DH))

    # causal triangle for the diagonal 128-blocks: keep sjl <= sil
    sjl = np.arange(128)[:, None]
    sil = np.arange(128)[None, :]
    trid = (sjl <= sil).astype(np.float32)

    # indicator for denominator broadcast: ind8[k, h*64+m] = (k == h)
    ind8 = np.zeros((8, 512), dtype=np.float32)
    for h in range(8):
        ind8[h, h * 64:(h + 1) * 64] = 1.0

    in_maps = []
    for c in range(NCORES):
        b, t = divmod(c, TP)
        e0 = t * E
        in_maps.append({
            "xT": np.ascontiguousarray(x[b].T).astype(BF16),
            "wqT": np.ascontiguousarray(wq_n[e0:e0 + E].T).astype(BF16),
            "wkT": np.ascontiguousarray(wk_n[e0:e0 + E].T).astype(BF16),
            "wvT": np.ascontiguousarray(wv_n[e0:e0 + E].T).astype(BF16),
            "woT": np.ascontiguousarray(wo_n[:, e0:e0 + E].T).astype(BF16),
            "cosd": cosd.astype(BF16), "sind": sind.astype(BF16),
            "trid": trid.astype(BF16), "ind8d": ind8.astype(BF16),
        })
    return in_maps


def _install_profile_hook():
    """antenv.axon_hooks is absent in this image; shim it and register the
    ctypes NTFF hook against /opt/axon/libaxon_pjrt.so (mirrors trn_boot)."""
    import types
    import ctypes
    import contextlib

    try:
        from antenv.axon_hooks import get_axon_ntff_profile_hook  # noqa
        return
    except ImportError:
        pass
    import antenv
    mod = types.ModuleType("antenv.axon_hooks")
    state = {}
    mod.set_axon_ntff_profile_hook = lambda h: state.__setitem__("h", h)
    mod.get_axon_ntff_profile_hook = lambda: state.get("h")
    sys.modules["antenv.axon_hooks"] = mod
    antenv.axon_hooks = mod

    so_path = "/opt/axon/libaxon_pjrt.so"
    lib = ctypes.CDLL(so_path)
    if not hasattr(lib, "axon_start_nrt_profile"):
        return
    lib.axon_start_nrt_profile.argtypes = [
        ctypes.POINTER(ctypes.c_int64), ctypes.c_size_t]
    lib.axon_start_nrt_profile.restype = ctypes.c_int64
    lib.axon_stop_nrt_profile.argtypes = [ctypes.c_char_p]
    lib.axon_stop_nrt_profile.restype = ctypes.c_int64

    @contextlib.contextmanager
    def _hook(output_dir, device_ids):
        import jax
        jax.devices()
        if device_ids:
            ids = (ctypes.c_int64 * len(device_ids))(*device_ids)
            rc = lib.axon_start_nrt_profile(ids, len(device_ids))
        else:
            rc = lib.axon_start_nrt_profile(None, 0)
        if rc != 0:
            raise RuntimeError(f"axon_start_nrt_profile rc={rc}")
        try:
            yield
        finally:
            n = lib.axon_stop_nrt_profile(str(output_dir).encode())
            print(f"profile: {n} file(s) written to {output_dir}",
                  file=sys.stderr)

    mod.set_axon_ntff_profile_hook(_hook)


def kernel(x, wq, wk, wv, wo, qk_scale, _profile=False):
    from concourse.bass_utils import run_bass_kernel_spmd

    if _profile:
        _install_profile_hook()

    if "nc" not in _CACHE:
        nc = _build_program()
        nc.finalize()
        _CACHE["nc"] = nc
    nc = _CACHE["nc"]
    in_maps = _host_prep(np.asarray(x), np.asarray(wq), np.asarray(wk),
                         np.asarray(wv), np.asarray(wo), np.asarray(qk_scale))
    res = run_bass_kernel_spmd(nc, in_maps, core_ids=list(range(NCORES)),
                               trace=_profile)
    outs = res.results
    y = np.empty((B, S, DIM), dtype=np.float32)
    for b in range(B):
        y[b] = sum(outs[b * TP + t]["Y"] for t in range(TP))
    if _profile:
        _CACHE["last_exec_time_ns"] = res.exec_time_ns
        _CACHE["last_profile"] = res.profile_json
    return y


# revision 16
# speedup vs baseline: 1.0129x; 1.0129x over previous
"""nn_Attention Trainium2 Bass kernel (v2 — interleaved pipeline).

Full attention forward: x->(q,k,v) with l2-normalized weights, per-head-dim
l2 norm + learned qk scale, interleaved RoPE, causal SDPA, output projection
with column-l2-normalized wo.

Sharding: TP=4 over heads (8 heads/core) x DP=2 over batch across 8 cores.
Each core computes a partial [2048, 2048] output for its batch; host sums
the 4 TP partials per batch.

v2 changes vs v1:
- single interleaved loop per 512-row block: proj -> attention -> yproj,
  so DVE rope work, Act exp work and PE matmuls overlap across phases.
- q/k transposes via DMA xbar (dma_start_transpose) instead of PE
  transposes + DVE copies.
- causal mask as a single 128x128 triangle multiply on the Pool engine.
- lg/pv matmuls trimmed to the live columns on diagonal blocks.
- softmax denominators: v's 65th ones-column -> psum row 64 -> stashed ->
  gathered by DMA -> PE-transposed to si-partition layout -> one cheap
  [128,32] reciprocal -> transposed back -> rank-8 indicator matmul
  broadcast (replaces 3.3us-per-call wide DVE reciprocals).
- x streamed per 512-column block (2-deep) instead of fully resident.
- yproj results DMA'd directly from PSUM to DRAM.
"""
import sys
import os
import math
from contextlib import ExitStack

sys.path.insert(0, "/opt/trn_rl_repo")

import numpy as np
import ml_dtypes

BF16 = ml_dtypes.bfloat16

B, S, DIM = 2, 2048, 2048
HEADS, DH = 32, 64
THETA = 10000.0
NCORES = 8
TP = 4             # head-parallel ways
HPC = HEADS // TP  # heads per core = 8
E = HPC * DH       # per-core qkv width = 512
ET = E // 128      # e-tiles per core = 4
DT = DIM // 128    # contraction d-tiles = 16
SB = S // 512      # 512-wide seq blocks = 4
SS = S // 128      # 128-wide seq blocks = 16

_CACHE = {}


def _l2n(w, axis):
    n = np.sqrt((w.astype(np.float64) ** 2).sum(axis=axis, keepdims=True))
    n = np.maximum(n, 1e-12)
    return (w / n).astype(np.float32)


def _build_program():
    import concourse.bass as bass
    from concourse import bacc
    import concourse.mybir as mybir
    import concourse.tile as tile
    from concourse.masks import make_identity

    f32 = mybir.dt.float32
    bf16 = mybir.dt.bfloat16
    AF = mybir.ActivationFunctionType
    AX = mybir.AxisListType
    OP = mybir.AluOpType

    nc = bacc.Bacc("TRN2", target_bir_lowering=False)

    xT = nc.dram_tensor("xT", [DIM, S], bf16, kind="ExternalInput")
    wqT = nc.dram_tensor("wqT", [DIM, E], bf16, kind="ExternalInput")
    wkT = nc.dram_tensor("wkT", [DIM, E], bf16, kind="ExternalInput")
    wvT = nc.dram_tensor("wvT", [DIM, E], bf16, kind="ExternalInput")
    woT = nc.dram_tensor("woT", [E, DIM], bf16, kind="ExternalInput")
    cosd = nc.dram_tensor("cosd", [128, SS * DH], bf16, kind="ExternalInput")
    sind = nc.dram_tensor("sind", [128, SS * DH], bf16, kind="ExternalInput")
    trid = nc.dram_tensor("trid", [128, 128], bf16, kind="ExternalInput")
    ind8d = nc.dram_tensor("ind8d", [8, 512], bf16, kind="ExternalInput")
    Y = nc.dram_tensor("Y", [S, DIM], f32, kind="ExternalOutput")

    with tile.TileContext(nc) as tc, ExitStack() as ctx:
        const = ctx.enter_context(tc.tile_pool(name="const", bufs=1))
        wpool = ctx.enter_context(tc.tile_pool(name="wpool", bufs=4))
        xpool = ctx.enter_context(tc.tile_pool(name="xpool", bufs=2))
        qkv = ctx.enter_context(tc.tile_pool(name="qkv", bufs=1))
        work = ctx.enter_context(tc.tile_pool(name="work", bufs=1))
        expool = ctx.enter_context(tc.tile_pool(name="expool", bufs=4))
        psA = ctx.enter_context(
            tc.tile_pool(name="psA", bufs=4, space="PSUM"))
        psL = ctx.enter_context(
            tc.tile_pool(name="psL", bufs=2, space="PSUM"))

        # --- weights (wq first, quartered, so proj can start early) ---
        wq_sb = [wpool.tile([128, 4, E], bf16, tag=f"wq{j}", bufs=1, name=f"wq{j}")
                 for j in range(4)]
        wk_sb = wpool.tile([128, DT, E], bf16, tag="wk", bufs=1)
        wv_sb = wpool.tile([128, DT, E], bf16, tag="wv", bufs=1)
        wo_sb = wpool.tile([128, ET, DIM], bf16, tag="wo", bufs=1)
        wqr = wqT.rearrange("(t p) e -> p t e", p=128)

        xtiles = {}

        def load_x(st):
            ts = [xpool.tile([128, 4, 512], bf16, tag=f"x{j}", bufs=2,
                             name=f"xst{st}_{j}") for j in range(4)]
            src = xT[:, st * 512:(st + 1) * 512].rearrange(
                "(t p) s -> p t s", p=128)
            for j in range(4):
                nc.sync.dma_start(ts[j], src[:, j * 4:(j + 1) * 4, :])
            return ts

        # interleave wq quarters with x quarters so dt=0..3 can start early
        x0src = xT[:, 0:512].rearrange("(t p) s -> p t s", p=128)
        x0 = [xpool.tile([128, 4, 512], bf16, tag=f"x{j}", bufs=2,
                         name=f"xst0_{j}") for j in range(4)]
        for j in range(4):
            nc.sync.dma_start(wq_sb[j], wqr[:, j * 4:(j + 1) * 4, :])
            nc.sync.dma_start(x0[j], x0src[:, j * 4:(j + 1) * 4, :])
        xtiles[0] = x0
        nc.sync.dma_start(wk_sb, wkT.rearrange("(t p) e -> p t e", p=128))
        nc.sync.dma_start(wv_sb, wvT.rearrange("(t p) e -> p t e", p=128))

        # --- constants ---
        cos_sb = const.tile([128, SS, DH], bf16)
        sin_sb = const.tile([128, SS, DH], bf16)
        nc.sync.dma_start(cos_sb, cosd.rearrange("p (b d) -> p b d", d=DH))
        nc.sync.dma_start(sin_sb, sind.rearrange("p (b d) -> p b d", d=DH))
        tri = const.tile([128, 128], bf16)
        nc.sync.dma_start(tri, trid[:, :])
        ind8 = const.tile([8, 512], bf16)
        nc.sync.dma_start(ind8, ind8d[:, :])
        nc.sync.dma_start(wo_sb, woT.rearrange("(t p) e -> p t e", p=128))
        identf = const.tile([128, 128], f32)
        make_identity(nc, identf)
        ident = const.tile([128, 128], bf16)
        make_identity(nc, ident)

        # --- persistent activations ---
        qTall = qkv.tile([128, ET, S], bf16, tag="qT")
        kTall = qkv.tile([128, ET, S], bf16, tag="kT")
        v_sb = qkv.tile([128, SS, HPC, 65], bf16, tag="v")
        stash = qkv.tile([65, HPC, 512], bf16, tag="stash")
        nc.vector.memset(v_sb[:, :, :, 64:65], 1.0)

        def norm_rope(ps, dstT, st, su):
            """psum [si,e] natural -> per-head l2norm, rope, bf16,
            -> DMA-transpose into dstT columns."""
            sblk = st * 4 + su
            sq = work.tile([128, E], bf16, tag="sq", bufs=2)
            nc.scalar.square(sq, ps)
            ssq = work.tile([128, HPC], f32, tag="ssq", bufs=2)
            nc.vector.tensor_reduce(
                ssq, sq.rearrange("p (h d) -> p h d", d=DH),
                axis=AX.X, op=OP.add)
            nc.scalar.sqrt(ssq, ssq)
            inv = work.tile([128, HPC], f32, tag="inv", bufs=2)
            nc.vector.reciprocal(inv, ssq)
            qn = work.tile([128, HPC, DH], bf16, tag="qn", bufs=2)
            nc.vector.tensor_mul(
                qn, ps.rearrange("p (h d) -> p h d", d=DH),
                inv.unsqueeze(2).broadcast_to([128, HPC, DH]))
            cosb = cos_sb[:, sblk:sblk + 1, :].broadcast_to([128, HPC, DH])
            sinb = sin_sb[:, sblk:sblk + 1, :].broadcast_to([128, HPC, DH])
            rot = work.tile([128, HPC, 2, 32], bf16, tag="rot", bufs=2)
            qn4 = qn.rearrange("p h (t u) -> p h t u", u=32)
            nc.vector.tensor_copy(rot[:, :, 0:1, :], qn4[:, :, 1:2, :])
            nc.vector.tensor_copy(rot[:, :, 1:2, :], qn4[:, :, 0:1, :])
            nc.vector.tensor_mul(rot.rearrange("p h t u -> p h (t u)"),
                                 rot.rearrange("p h t u -> p h (t u)"), sinb)
            nc.vector.tensor_mul(qn, qn, cosb)
            qo = work.tile([128, E], bf16, tag="qo", bufs=2)
            nc.vector.tensor_add(
                qo, qn.rearrange("p h d -> p (h d)"),
                rot.rearrange("p h t u -> p (h t u)"))
            nc.sync.dma_start_transpose(
                dstT[:, :, sblk * 128:(sblk + 1) * 128], qo)

        def proj_wave(w_sb, kind, st, xt):
            quartered = isinstance(w_sb, list)
            for su in range(4):
                ps = psA.tile([128, E], f32, tag="ps",
                              name=f"p{kind}{st}_{su}")
                for dt in range(DT):
                    if quartered:
                        wslice = w_sb[dt // 4][:, dt % 4, :]
                    else:
                        wslice = w_sb[:, dt, :]
                    nc.tensor.matmul(
                        ps,
                        xt[dt // 4][:, dt % 4, su * 128:(su + 1) * 128],
                        wslice,
                        start=(dt == 0), stop=(dt == DT - 1))
                if kind == "v":
                    nc.vector.tensor_copy(
                        v_sb[:, st * 4 + su, :, 0:64],
                        ps.rearrange("p (h d) -> p h d", d=DH))
                else:
                    norm_rope(ps, qTall if kind == "q" else kTall, st, su)

        def proj_all(st):
            xt = xtiles[st]
            proj_wave(wq_sb, "q", st, xt)
            proj_wave(wk_sb, "k", st, xt)
            proj_wave(wv_sb, "v", st, xt)

        def attn_block(i):
            last = 4 * i + 3
            for h in range(HPC):
                et, hp = h // 2, (h % 2) * 64
                pv = psA.tile([128, 512], f32, tag="ps", name=f"pv{i}_{h}")
                npr = 2 * (i + 1)
                lgs = {}

                def emit_lg(p):
                    lg2 = psL.tile([128, 2, 512], f32, tag="lg",
                                   name=f"lg{i}_{h}_{p}")
                    for b in range(2):
                        sjb = 2 * p + b
                        r = sjb - 4 * i
                        c0 = r * 128 if r > 0 else 0
                        nc.tensor.matmul(
                            lg2[:, b, c0:],
                            kTall[hp:hp + 64, et, sjb * 128:(sjb + 1) * 128],
                            qTall[hp:hp + 64, et,
                                  i * 512 + c0:(i + 1) * 512],
                            start=True, stop=True)
                    lgs[p] = lg2

                emit_lg(0)
                if npr > 1:
                    emit_lg(1)
                for p in range(npr):
                    lg2 = lgs.pop(p)
                    ex = expool.tile([128, 2, 512], bf16, tag="ex")
                    if 2 * p - 4 * i >= 0:  # diagonal pair: match lg trim
                        for b in range(2):
                            c0 = max(0, (2 * p + b - 4 * i)) * 128
                            nc.scalar.activation(ex[:, b, c0:],
                                                 lg2[:, b, c0:], AF.Exp)
                    else:
                        nc.scalar.activation(ex, lg2, AF.Exp)
                    if p + 2 < npr:
                        emit_lg(p + 2)
                    for b in range(2):
                        sjb = 2 * p + b
                        r = sjb - 4 * i
                        if r >= 0:
                            nc.gpsimd.tensor_mul(
                                ex[:, b, r * 128:(r + 1) * 128],
                                ex[:, b, r * 128:(r + 1) * 128], tri)
                        c0 = r * 128 if r > 0 else 0
                        nc.tensor.matmul(
                            pv[0:65, c0:],
                            v_sb[:, sjb, h, :],
                            ex[:, b, c0:],
                            start=(sjb == 0), stop=(sjb == last))
                nc.vector.tensor_copy(stash[:, h, :], pv[0:65, :])

        def normalize_gather(i):
            den = work.tile([8, 512], bf16, tag="den", bufs=2)
            nc.scalar.dma_start(den, stash[64:65, :, :])
            return den

        def normalize_recip(i, den):
            """den rows -> si-partition layout via PE transposes -> one
            cheap [128,32] DVE reciprocal."""
            invT = psA.tile([128, 32], bf16, tag="ps")
            for c in range(4):
                nc.tensor.transpose(
                    invT[:, c * 8:(c + 1) * 8],
                    den[:, c * 128:(c + 1) * 128], ident[0:8, 0:8])
            inv_sb = work.tile([128, 32], f32, tag="invsb", bufs=2)
            nc.vector.reciprocal(inv_sb, invT)
            return inv_sb

        def normalize_apply(i, inv_sb):
            """transpose back to row layout, rank-8 indicator broadcast,
            per-head mul into qTall."""
            invrow = psA.tile([8, 4, 128], f32, tag="ps")
            for c in range(4):
                nc.tensor.transpose(
                    invrow[:, c, :], inv_sb[:, c * 8:(c + 1) * 8], identf)
            inv_row = work.tile([8, 512], bf16, tag="invrowsb", bufs=2)
            nc.vector.tensor_copy(
                inv_row, invrow.rearrange("p c j -> p (c j)"))
            for h in range(HPC):
                et, hp = h // 2, (h % 2) * 64
                bc = psA.tile([64, 512], f32, tag="ps", name=f"bc{i}_{h}")
                nc.tensor.matmul(bc, ind8[:, h * 64:(h + 1) * 64], inv_row,
                                 start=True, stop=True)
                nc.vector.tensor_mul(
                    qTall[hp:hp + 64, et, i * 512:(i + 1) * 512],
                    stash[0:64, h, :], bc)

        def yproj_block(i):
            for ib in range(4 * i, 4 * i + 4):
                for nd in range(4):
                    ps = psA.tile([128, 512], f32, tag="ps",
                                  name=f"y{ib}_{nd}")
                    for ket in range(ET):
                        nc.tensor.matmul(
                            ps,
                            qTall[:, ket, ib * 128:(ib + 1) * 128],
                            wo_sb[:, ket, nd * 512:(nd + 1) * 512],
                            start=(ket == 0), stop=(ket == ET - 1))
                    ys = work.tile([128, 512], f32, tag="ys", bufs=2)
                    if nd % 2 == 0:
                        nc.vector.tensor_copy(ys, ps)
                    else:
                        nc.scalar.copy(ys, ps)
                    nc.sync.dma_start(
                        Y[ib * 128:(ib + 1) * 128, nd * 512:(nd + 1) * 512],
                        ys)

        proj_all(0)
        for st in range(SB):
            if st + 1 < SB:
                xtiles[st + 1] = load_x(st + 1)
            attn_block(st)
            den = normalize_gather(st)
            inv_sb = normalize_recip(st, den)
            if st + 1 < SB:
                proj_wave(wq_sb, "q", st + 1, xtiles[st + 1])
                normalize_apply(st, inv_sb)
                proj_wave(wk_sb, "k", st + 1, xtiles[st + 1])
                proj_wave(wv_sb, "v", st + 1, xtiles[st + 1])
            else:
                normalize_apply(st, inv_sb)
            yproj_block(st)

    return nc


def _host_prep(x, wq, wk, wv, wo, qk_scale):
    """Returns per-core input dicts."""
    perm = np.concatenate([np.arange(0, DH, 2), np.arange(1, DH, 2)])
    wq_n = _l2n(wq, -1).reshape(HEADS, DH, DIM)[:, perm, :].reshape(HEADS * DH, DIM)
    wk_n = _l2n(wk, -1).reshape(HEADS, DH, DIM)[:, perm, :].reshape(HEADS * DH, DIM)
    wv_n = _l2n(wv, -1)
    wo_n = _l2n(wo, 0)
    sp = qk_scale.astype(np.float64)[perm]

    # rope tables with qk_scale folded in; permuted-block layout
    half = np.arange(0, DH, 2)
    freqs = 1.0 / (THETA ** (half.astype(np.float64) / DH))      # (32,)
    ang = np.arange(S, dtype=np.float64)[:, None] * freqs[None]  # (S, 32)
    cos_h, sin_h = np.cos(ang), np.sin(ang)
    cos_p = np.concatenate([cos_h, cos_h], 1)                    # (S, 64)
    sin_e = np.concatenate([-sin_h, sin_h], 1)
    cos_eff = (cos_p * sp[None, :]).astype(np.float32)
    swap_sp = np.concatenate([sp[32:], sp[:32]])
    sin_eff = (sin_e * swap_sp[None, :]).astype(np.float32)
    # device layout [128, SS*DH]: [p, b*64+c] = tbl[b*128+p, c]
    cosd = np.ascontiguousarray(
        cos_eff.reshape(SS, 128, DH).transpose(1, 0, 2).reshape(128, SS * DH))
    sind = np.ascontiguousarray(
        sin_eff.reshape(SS, 128, DH).transpose(1, 0, 2).reshape(128, SS * DH))

    # causal triangle for the diagonal 128-blocks: keep sjl <= sil
    sjl = np.arange(128)[:, None]
    sil = np.arange(128)[None, :]
    trid = (sjl <= sil).astype(np.float32)

    # indicator for denominator broadcast: ind8[k, h*64+m] = (k == h)
    ind8 = np.zeros((8, 512), dtype=np.float32)
    for h in range(8):
        ind8[h, h * 64:(h + 1) * 64] = 1.0

    in_maps = []
    for c in range(NCORES):
        b, t = divmod(c, TP)
        e0 = t * E
        in_maps.append({
            "xT": np.ascontiguousarray(x[b].T).astype(BF16),
            "wqT": np.ascontiguousarray(wq_n[e0:e0 + E].T).astype(BF16),
            "wkT": np.ascontiguousarray(wk_n[e0:e0 + E].T).astype(BF16),
            "wvT": np.ascontiguousarray(wv_n[e0:e0 + E].T).astype(BF16),
            "woT": np.ascontiguousarray(wo_n[:, e0:e0 + E].T).astype(BF16),
            "cosd": cosd.astype(BF16), "sind": sind.astype(BF16),
            "trid": trid.astype(BF16), "ind8d": ind8.astype(BF16),
        })
    return in_maps


def _install_profile_hook():
    """antenv.axon_hooks is absent in this image; shim it and register the
    ctypes NTFF hook against /opt/axon/libaxon_pjrt.so (mirrors trn_boot)."""
    import types
    import ctypes
    import contextlib

    try:
        from antenv.axon_hooks import get_axon_ntff_profile_hook  # noqa
        return
    except ImportError:
        pass
    import antenv
    mod = types.ModuleType("antenv.axon_hooks")
    state = {}
    mod.set_axon_ntff_profile_hook = lambda h: state.__setitem__("h", h)
    mod.get_axon_ntff_profile_hook = lambda: state.get("h")
    sys.modules["antenv.axon_hooks"] = mod
    antenv.axon_hooks = mod

    so_path = "/opt/axon/libaxon_pjrt.so"
    lib = ctypes.CDLL(so_path)
    if not hasattr(lib, "axon_start_nrt_profile"):
        return
    lib.axon_start_nrt_profile.argtypes = [
        ctypes.POINTER(ctypes.c_int64), ctypes.c_size_t]
    lib.axon_start_nrt_profile.restype = ctypes.c_int64
    lib.axon_stop_nrt_profile.argtypes = [ctypes.c_char_p]
    lib.axon_stop_nrt_profile.restype = ctypes.c_int64

    @contextlib.contextmanager
    def _hook(output_dir, device_ids):
        import jax
        jax.devices()
        if device_ids:
            ids = (ctypes.c_int64 * len(device_ids))(*device_ids)
            rc = lib.axon_start_nrt_profile(ids, len(device_ids))
        else:
            rc = lib.axon_start_nrt_profile(None, 0)
        if rc != 0:
            raise RuntimeError(f"axon_start_nrt_profile rc={rc}")
        try:
            yield
        finally:
            n = lib.axon_stop_nrt_profile(str(output_dir).encode())
            print(f"profile: {n} file(s) written to {output_dir}",
                  file=sys.stderr)

    mod.set_axon_ntff_profile_hook(_hook)


def kernel(x, wq, wk, wv, wo, qk_scale, _profile=False):
    from concourse.bass_utils import run_bass_kernel_spmd

    if _profile:
        _install_profile_hook()

    if "nc" not in _CACHE:
        nc = _build_program()
        nc.finalize()
        _CACHE["nc"] = nc
    nc = _CACHE["nc"]
    in_maps = _host_prep(np.asarray(x), np.asarray(wq), np.asarray(wk),
                         np.asarray(wv), np.asarray(wo), np.asarray(qk_scale))
    res = run_bass_kernel_spmd(nc, in_maps, core_ids=list(range(NCORES)),
                               trace=_profile)
    outs = res.results
    y = np.empty((B, S, DIM), dtype=np.float32)
    for b in range(B):
        y[b] = sum(outs[b * TP + t]["Y"] for t in range(TP))
    if _profile:
        _CACHE["last_exec_time_ns"] = res.exec_time_ns
        _CACHE["last_profile"] = res.profile_json
    return y


# revision 17
# speedup vs baseline: 1.0630x; 1.0495x over previous
"""nn_Attention Trainium2 Bass kernel (v2 — interleaved pipeline).

Full attention forward: x->(q,k,v) with l2-normalized weights, per-head-dim
l2 norm + learned qk scale, interleaved RoPE, causal SDPA, output projection
with column-l2-normalized wo.

Sharding: TP=4 over heads (8 heads/core) x DP=2 over batch across 8 cores.
Each core computes a partial [2048, 2048] output for its batch; host sums
the 4 TP partials per batch.

v2 changes vs v1:
- single interleaved loop per 512-row block: proj -> attention -> yproj,
  so DVE rope work, Act exp work and PE matmuls overlap across phases.
- q/k transposes via DMA xbar (dma_start_transpose) instead of PE
  transposes + DVE copies.
- causal mask as a single 128x128 triangle multiply on the Pool engine.
- lg/pv matmuls trimmed to the live columns on diagonal blocks.
- softmax denominators: v's 65th ones-column -> psum row 64 -> stashed ->
  gathered by DMA -> PE-transposed to si-partition layout -> one cheap
  [128,32] reciprocal -> transposed back -> rank-8 indicator matmul
  broadcast (replaces 3.3us-per-call wide DVE reciprocals).
- x streamed per 512-column block (2-deep) instead of fully resident.
- yproj results DMA'd directly from PSUM to DRAM.
"""
import sys
import os
import math
from contextlib import ExitStack

sys.path.insert(0, "/opt/trn_rl_repo")

import numpy as np
import ml_dtypes

BF16 = ml_dtypes.bfloat16

B, S, DIM = 2, 2048, 2048
HEADS, DH = 32, 64
THETA = 10000.0
NCORES = 8
TP = 4             # head-parallel ways
HPC = HEADS // TP  # heads per core = 8
E = HPC * DH       # per-core qkv width = 512
ET = E // 128      # e-tiles per core = 4
DT = DIM // 128    # contraction d-tiles = 16
SB = S // 512      # 512-wide seq blocks = 4
SS = S // 128      # 128-wide seq blocks = 16

_CACHE = {}


def _l2n(w, axis):
    n = np.sqrt((w.astype(np.float64) ** 2).sum(axis=axis, keepdims=True))
    n = np.maximum(n, 1e-12)
    return (w / n).astype(np.float32)


def _build_program():
    import concourse.bass as bass
    from concourse import bacc
    import concourse.mybir as mybir
    import concourse.tile as tile
    from concourse.masks import make_identity

    f32 = mybir.dt.float32
    bf16 = mybir.dt.bfloat16
    AF = mybir.ActivationFunctionType
    AX = mybir.AxisListType
    OP = mybir.AluOpType

    nc = bacc.Bacc("TRN2", target_bir_lowering=False)

    xT = nc.dram_tensor("xT", [DIM, S], bf16, kind="ExternalInput")
    wqT = nc.dram_tensor("wqT", [DIM, E], bf16, kind="ExternalInput")
    wkT = nc.dram_tensor("wkT", [DIM, E], bf16, kind="ExternalInput")
    wvT = nc.dram_tensor("wvT", [DIM, E], bf16, kind="ExternalInput")
    woT = nc.dram_tensor("woT", [E, DIM], bf16, kind="ExternalInput")
    cosd = nc.dram_tensor("cosd", [128, SS * DH], bf16, kind="ExternalInput")
    sind = nc.dram_tensor("sind", [128, SS * DH], bf16, kind="ExternalInput")
    trid = nc.dram_tensor("trid", [128, 128], bf16, kind="ExternalInput")
    ind8d = nc.dram_tensor("ind8d", [8, 512], bf16, kind="ExternalInput")
    Y = nc.dram_tensor("Y", [S, DIM], f32, kind="ExternalOutput")

    with tile.TileContext(nc) as tc, ExitStack() as ctx:
        const = ctx.enter_context(tc.tile_pool(name="const", bufs=1))
        wpool = ctx.enter_context(tc.tile_pool(name="wpool", bufs=4))
        xpool = ctx.enter_context(tc.tile_pool(name="xpool", bufs=2))
        qkv = ctx.enter_context(tc.tile_pool(name="qkv", bufs=1))
        work = ctx.enter_context(tc.tile_pool(name="work", bufs=1))
        expool = ctx.enter_context(tc.tile_pool(name="expool", bufs=4))
        psA = ctx.enter_context(
            tc.tile_pool(name="psA", bufs=4, space="PSUM"))
        psL = ctx.enter_context(
            tc.tile_pool(name="psL", bufs=2, space="PSUM"))

        # --- weights (wq first, quartered, so proj can start early) ---
        wq_sb = [wpool.tile([128, 4, E], bf16, tag=f"wq{j}", bufs=1, name=f"wq{j}")
                 for j in range(4)]
        wk_sb = wpool.tile([128, DT, E], bf16, tag="wk", bufs=1)
        wv_sb = wpool.tile([128, DT, E], bf16, tag="wv", bufs=1)
        wo_sb = wpool.tile([128, ET, DIM], bf16, tag="wo", bufs=1)
        wqr = wqT.rearrange("(t p) e -> p t e", p=128)

        xtiles = {}

        def load_x(st):
            ts = [xpool.tile([128, 4, 512], bf16, tag=f"x{j}", bufs=2,
                             name=f"xst{st}_{j}") for j in range(4)]
            src = xT[:, st * 512:(st + 1) * 512].rearrange(
                "(t p) s -> p t s", p=128)
            for j in range(4):
                nc.sync.dma_start(ts[j], src[:, j * 4:(j + 1) * 4, :])
            return ts

        # interleave wq quarters with x quarters so dt=0..3 can start early
        x0src = xT[:, 0:512].rearrange("(t p) s -> p t s", p=128)
        x0 = [xpool.tile([128, 4, 512], bf16, tag=f"x{j}", bufs=2,
                         name=f"xst0_{j}") for j in range(4)]
        for j in range(4):
            nc.sync.dma_start(wq_sb[j], wqr[:, j * 4:(j + 1) * 4, :])
            nc.sync.dma_start(x0[j], x0src[:, j * 4:(j + 1) * 4, :])
        xtiles[0] = x0
        nc.sync.dma_start(wk_sb, wkT.rearrange("(t p) e -> p t e", p=128))
        nc.sync.dma_start(wv_sb, wvT.rearrange("(t p) e -> p t e", p=128))

        # --- constants ---
        cos_sb = const.tile([128, SS, DH], bf16)
        sin_sb = const.tile([128, SS, DH], bf16)
        nc.sync.dma_start(cos_sb, cosd.rearrange("p (b d) -> p b d", d=DH))
        nc.sync.dma_start(sin_sb, sind.rearrange("p (b d) -> p b d", d=DH))
        tri = const.tile([128, 128], bf16)
        nc.sync.dma_start(tri, trid[:, :])
        ind8 = const.tile([8, 512], bf16)
        nc.sync.dma_start(ind8, ind8d[:, :])
        nc.sync.dma_start(wo_sb, woT.rearrange("(t p) e -> p t e", p=128))
        identf = const.tile([128, 128], f32)
        make_identity(nc, identf)
        ident = const.tile([128, 128], bf16)
        make_identity(nc, ident)

        # --- persistent activations ---
        qTall = qkv.tile([128, ET, S], bf16, tag="qT")
        kTall = qkv.tile([128, ET, S], bf16, tag="kT")
        v_sb = qkv.tile([128, SS, HPC, 65], bf16, tag="v")
        stash = qkv.tile([65, HPC, 512], bf16, tag="stash")
        nc.vector.memset(v_sb[:, :, :, 64:65], 1.0)

        def norm_rope(ps, dstT, st, su):
            """psum [si,e] natural -> per-head l2norm, rope, bf16,
            -> DMA-transpose into dstT columns."""
            sblk = st * 4 + su
            sq = work.tile([128, E], bf16, tag="sq", bufs=2)
            nc.scalar.square(sq, ps)
            ssq = work.tile([128, HPC], f32, tag="ssq", bufs=2)
            nc.vector.tensor_reduce(
                ssq, sq.rearrange("p (h d) -> p h d", d=DH),
                axis=AX.X, op=OP.add)
            nc.scalar.sqrt(ssq, ssq)
            inv = work.tile([128, HPC], f32, tag="inv", bufs=2)
            nc.vector.reciprocal(inv, ssq)
            qn = work.tile([128, HPC, DH], bf16, tag="qn", bufs=2)
            nc.vector.tensor_mul(
                qn, ps.rearrange("p (h d) -> p h d", d=DH),
                inv.unsqueeze(2).broadcast_to([128, HPC, DH]))
            cosb = cos_sb[:, sblk:sblk + 1, :].broadcast_to([128, HPC, DH])
            sinb = sin_sb[:, sblk:sblk + 1, :].broadcast_to([128, HPC, DH])
            rot = work.tile([128, HPC, 2, 32], bf16, tag="rot", bufs=2)
            qn4 = qn.rearrange("p h (t u) -> p h t u", u=32)
            nc.vector.tensor_copy(rot[:, :, 0:1, :], qn4[:, :, 1:2, :])
            nc.vector.tensor_copy(rot[:, :, 1:2, :], qn4[:, :, 0:1, :])
            nc.vector.tensor_mul(rot.rearrange("p h t u -> p h (t u)"),
                                 rot.rearrange("p h t u -> p h (t u)"), sinb)
            nc.vector.tensor_mul(qn, qn, cosb)
            qo = work.tile([128, E], bf16, tag="qo", bufs=2)
            nc.vector.tensor_add(
                qo, qn.rearrange("p h d -> p (h d)"),
                rot.rearrange("p h t u -> p (h t u)"))
            nc.sync.dma_start_transpose(
                dstT[:, :, sblk * 128:(sblk + 1) * 128], qo)

        def proj_wave(w_sb, kind, st, xt):
            quartered = isinstance(w_sb, list)
            for s0 in (0, 2):
                prs = [psA.tile([128, E], f32, tag="ps",
                                name=f"p{kind}{st}_{s0 + j}")
                       for j in range(2)]
                for dt in range(DT):
                    if quartered:
                        wslice = w_sb[dt // 4][:, dt % 4, :]
                    else:
                        wslice = w_sb[:, dt, :]
                    for j in range(2):
                        su = s0 + j
                        nc.tensor.matmul(
                            prs[j],
                            xt[dt // 4][:, dt % 4,
                                        su * 128:(su + 1) * 128],
                            wslice,
                            start=(dt == 0), stop=(dt == DT - 1))
                for j in range(2):
                    su = s0 + j
                    if kind == "v":
                        nc.vector.tensor_copy(
                            v_sb[:, st * 4 + su, :, 0:64],
                            prs[j].rearrange("p (h d) -> p h d", d=DH))
                    else:
                        norm_rope(prs[j], qTall if kind == "q" else kTall,
                                  st, su)

        def proj_all(st):
            xt = xtiles[st]
            proj_wave(wq_sb, "q", st, xt)
            proj_wave(wk_sb, "k", st, xt)
            proj_wave(wv_sb, "v", st, xt)

        def attn_block(i):
            last = 4 * i + 3
            for h in range(HPC):
                et, hp = h // 2, (h % 2) * 64
                pv = psA.tile([128, 512], f32, tag="ps", name=f"pv{i}_{h}")
                npr = 2 * (i + 1)
                lgs = {}

                def emit_lg_b(p, b):
                    if p not in lgs:
                        lgs[p] = psL.tile([128, 2, 512], f32, tag="lg",
                                          name=f"lg{i}_{h}_{p}")
                    sjb = 2 * p + b
                    r = sjb - 4 * i
                    c0 = r * 128 if r > 0 else 0
                    nc.tensor.matmul(
                        lgs[p][:, b, c0:],
                        kTall[hp:hp + 64, et, sjb * 128:(sjb + 1) * 128],
                        qTall[hp:hp + 64, et, i * 512 + c0:(i + 1) * 512],
                        start=True, stop=True)

                def emit_lg(p):
                    emit_lg_b(p, 0)
                    emit_lg_b(p, 1)

                emit_lg(0)
                if npr > 1:
                    emit_lg(1)
                for p in range(npr):
                    lg2 = lgs.pop(p)
                    ex = expool.tile([128, 2, 512], bf16, tag="ex")
                    if 2 * p - 4 * i >= 0:  # diagonal pair: match lg trim
                        for b in range(2):
                            c0 = max(0, (2 * p + b - 4 * i)) * 128
                            nc.scalar.activation(ex[:, b, c0:],
                                                 lg2[:, b, c0:], AF.Exp)
                    else:
                        nc.scalar.activation(ex, lg2, AF.Exp)
                    for b in range(2):
                        sjb = 2 * p + b
                        r = sjb - 4 * i
                        if r >= 0:
                            nc.gpsimd.tensor_mul(
                                ex[:, b, r * 128:(r + 1) * 128],
                                ex[:, b, r * 128:(r + 1) * 128], tri)
                        c0 = r * 128 if r > 0 else 0
                        nc.tensor.matmul(
                            pv[0:65, c0:],
                            v_sb[:, sjb, h, :],
                            ex[:, b, c0:],
                            start=(sjb == 0), stop=(sjb == last))
                        if p + 2 < npr:
                            emit_lg_b(p + 2, b)
                nc.vector.tensor_copy(stash[:, h, :], pv[0:65, :])

        def normalize_gather(i):
            den = work.tile([8, 512], bf16, tag="den", bufs=2)
            nc.scalar.dma_start(den, stash[64:65, :, :])
            return den

        def normalize_recip(i, den):
            """den rows -> si-partition layout via PE transposes -> one
            cheap [128,32] DVE reciprocal."""
            invT = psA.tile([128, 32], bf16, tag="ps")
            for c in range(4):
                nc.tensor.transpose(
                    invT[:, c * 8:(c + 1) * 8],
                    den[:, c * 128:(c + 1) * 128], ident[0:8, 0:8])
            inv_sb = work.tile([128, 32], f32, tag="invsb", bufs=2)
            nc.vector.reciprocal(inv_sb, invT)
            return inv_sb

        def normalize_apply(i, inv_sb):
            """transpose back to row layout, rank-8 indicator broadcast,
            per-head mul into qTall."""
            invrow = psA.tile([8, 4, 128], f32, tag="ps")
            for c in range(4):
                nc.tensor.transpose(
                    invrow[:, c, :], inv_sb[:, c * 8:(c + 1) * 8], identf)
            inv_row = work.tile([8, 512], bf16, tag="invrowsb", bufs=2)
            nc.vector.tensor_copy(
                inv_row, invrow.rearrange("p c j -> p (c j)"))
            for h in range(HPC):
                et, hp = h // 2, (h % 2) * 64
                bc = psA.tile([64, 512], f32, tag="ps", name=f"bc{i}_{h}")
                nc.tensor.matmul(bc, ind8[:, h * 64:(h + 1) * 64], inv_row,
                                 start=True, stop=True)
                nc.vector.tensor_mul(
                    qTall[hp:hp + 64, et, i * 512:(i + 1) * 512],
                    stash[0:64, h, :], bc)

        def yproj_block(i):
            for ib in range(4 * i, 4 * i + 4):
                for nd0 in (0, 2):
                    pss = [psA.tile([128, 512], f32, tag="ps",
                                    name=f"y{ib}_{nd0 + j}")
                           for j in range(2)]
                    for ket in range(ET):
                        for j in range(2):
                            nd = nd0 + j
                            nc.tensor.matmul(
                                pss[j],
                                qTall[:, ket, ib * 128:(ib + 1) * 128],
                                wo_sb[:, ket, nd * 512:(nd + 1) * 512],
                                start=(ket == 0), stop=(ket == ET - 1))
                    for j in range(2):
                        nd = nd0 + j
                        ys = work.tile([128, 512], f32, tag="ys", bufs=2)
                        if nd % 2 == 0:
                            nc.vector.tensor_copy(ys, pss[j])
                        else:
                            nc.scalar.copy(ys, pss[j])
                        nc.sync.dma_start(
                            Y[ib * 128:(ib + 1) * 128,
                              nd * 512:(nd + 1) * 512], ys)

        proj_all(0)
        for st in range(SB):
            if st + 1 < SB:
                xtiles[st + 1] = load_x(st + 1)
            attn_block(st)
            den = normalize_gather(st)
            inv_sb = normalize_recip(st, den)
            if st + 1 < SB:
                proj_wave(wq_sb, "q", st + 1, xtiles[st + 1])
                normalize_apply(st, inv_sb)
                proj_wave(wk_sb, "k", st + 1, xtiles[st + 1])
                proj_wave(wv_sb, "v", st + 1, xtiles[st + 1])
            else:
                normalize_apply(st, inv_sb)
            yproj_block(st)

    return nc


def _host_prep(x, wq, wk, wv, wo, qk_scale):
    """Returns per-core input dicts."""
    perm = np.concatenate([np.arange(0, DH, 2), np.arange(1, DH, 2)])
    wq_n = _l2n(wq, -1).reshape(HEADS, DH, DIM)[:, perm, :].reshape(HEADS * DH, DIM)
    wk_n = _l2n(wk, -1).reshape(HEADS, DH, DIM)[:, perm, :].reshape(HEADS * DH, DIM)
    wv_n = _l2n(wv, -1)
    wo_n = _l2n(wo, 0)
    sp = qk_scale.astype(np.float64)[perm]

    # rope tables with qk_scale folded in; permuted-block layout
    half = np.arange(0, DH, 2)
    freqs = 1.0 / (THETA ** (half.astype(np.float64) / DH))      # (32,)
    ang = np.arange(S, dtype=np.float64)[:, None] * freqs[None]  # (S, 32)
    cos_h, sin_h = np.cos(ang), np.sin(ang)
    cos_p = np.concatenate([cos_h, cos_h], 1)                    # (S, 64)
    sin_e = np.concatenate([-sin_h, sin_h], 1)
    cos_eff = (cos_p * sp[None, :]).astype(np.float32)
    swap_sp = np.concatenate([sp[32:], sp[:32]])
    sin_eff = (sin_e * swap_sp[None, :]).astype(np.float32)
    # device layout [128, SS*DH]: [p, b*64+c] = tbl[b*128+p, c]
    cosd = np.ascontiguousarray(
        cos_eff.reshape(SS, 128, DH).transpose(1, 0, 2).reshape(128, SS * DH))
    sind = np.ascontiguousarray(
        sin_eff.reshape(SS, 128, DH).transpose(1, 0, 2).reshape(128, SS * DH))

    # causal triangle for the diagonal 128-blocks: keep sjl <= sil
    sjl = np.arange(128)[:, None]
    sil = np.arange(128)[None, :]
    trid = (sjl <= sil).astype(np.float32)

    # indicator for denominator broadcast: ind8[k, h*64+m] = (k == h)
    ind8 = np.zeros((8, 512), dtype=np.float32)
    for h in range(8):
        ind8[h, h * 64:(h + 1) * 64] = 1.0

    in_maps = []
    for c in range(NCORES):
        b, t = divmod(c, TP)
        e0 = t * E
        in_maps.append({
            "xT": np.ascontiguousarray(x[b].T).astype(BF16),
            "wqT": np.ascontiguousarray(wq_n[e0:e0 + E].T).astype(BF16),
            "wkT": np.ascontiguousarray(wk_n[e0:e0 + E].T).astype(BF16),
            "wvT": np.ascontiguousarray(wv_n[e0:e0 + E].T).astype(BF16),
            "woT": np.ascontiguousarray(wo_n[:, e0:e0 + E].T).astype(BF16),
            "cosd": cosd.astype(BF16), "sind": sind.astype(BF16),
            "trid": trid.astype(BF16), "ind8d": ind8.astype(BF16),
        })
    return in_maps


def _install_profile_hook():
    """antenv.axon_hooks is absent in this image; shim it and register the
    ctypes NTFF hook against /opt/axon/libaxon_pjrt.so (mirrors trn_boot)."""
    import types
    import ctypes
    import contextlib

    try:
        from antenv.axon_hooks import get_axon_ntff_profile_hook  # noqa
        return
    except ImportError:
        pass
    import antenv
    mod = types.ModuleType("antenv.axon_hooks")
    state = {}
    mod.set_axon_ntff_profile_hook = lambda h: state.__setitem__("h", h)
    mod.get_axon_ntff_profile_hook = lambda: state.get("h")
    sys.modules["antenv.axon_hooks"] = mod
    antenv.axon_hooks = mod

    so_path = "/opt/axon/libaxon_pjrt.so"
    lib = ctypes.CDLL(so_path)
    if not hasattr(lib, "axon_start_nrt_profile"):
        return
    lib.axon_start_nrt_profile.argtypes = [
        ctypes.POINTER(ctypes.c_int64), ctypes.c_size_t]
    lib.axon_start_nrt_profile.restype = ctypes.c_int64
    lib.axon_stop_nrt_profile.argtypes = [ctypes.c_char_p]
    lib.axon_stop_nrt_profile.restype = ctypes.c_int64

    @contextlib.contextmanager
    def _hook(output_dir, device_ids):
        import jax
        jax.devices()
        if device_ids:
            ids = (ctypes.c_int64 * len(device_ids))(*device_ids)
            rc = lib.axon_start_nrt_profile(ids, len(device_ids))
        else:
            rc = lib.axon_start_nrt_profile(None, 0)
        if rc != 0:
            raise RuntimeError(f"axon_start_nrt_profile rc={rc}")
        try:
            yield
        finally:
            n = lib.axon_stop_nrt_profile(str(output_dir).encode())
            print(f"profile: {n} file(s) written to {output_dir}",
                  file=sys.stderr)

    mod.set_axon_ntff_profile_hook(_hook)


def kernel(x, wq, wk, wv, wo, qk_scale, _profile=False):
    from concourse.bass_utils import run_bass_kernel_spmd

    if _profile:
        _install_profile_hook()

    if "nc" not in _CACHE:
        nc = _build_program()
        nc.finalize()
        _CACHE["nc"] = nc
    nc = _CACHE["nc"]
    in_maps = _host_prep(np.asarray(x), np.asarray(wq), np.asarray(wk),
                         np.asarray(wv), np.asarray(wo), np.asarray(qk_scale))
    res = run_bass_kernel_spmd(nc, in_maps, core_ids=list(range(NCORES)),
                               trace=_profile)
    outs = res.results
    y = np.empty((B, S, DIM), dtype=np.float32)
    for b in range(B):
        y[b] = sum(outs[b * TP + t]["Y"] for t in range(TP))
    if _profile:
        _CACHE["last_exec_time_ns"] = res.exec_time_ns
        _CACHE["last_profile"] = res.profile_json
    return y


# revision 18
# speedup vs baseline: 1.0945x; 1.0296x over previous
"""nn_Attention Trainium2 Bass kernel (v2 — interleaved pipeline).

Full attention forward: x->(q,k,v) with l2-normalized weights, per-head-dim
l2 norm + learned qk scale, interleaved RoPE, causal SDPA, output projection
with column-l2-normalized wo.

Sharding: TP=4 over heads (8 heads/core) x DP=2 over batch across 8 cores.
Each core computes a partial [2048, 2048] output for its batch; host sums
the 4 TP partials per batch.

v2 changes vs v1:
- single interleaved loop per 512-row block: proj -> attention -> yproj,
  so DVE rope work, Act exp work and PE matmuls overlap across phases.
- q/k transposes via DMA xbar (dma_start_transpose) instead of PE
  transposes + DVE copies.
- causal mask as a single 128x128 triangle multiply on the Pool engine.
- lg/pv matmuls trimmed to the live columns on diagonal blocks.
- softmax denominators: v's 65th ones-column -> psum row 64 -> stashed ->
  gathered by DMA -> PE-transposed to si-partition layout -> one cheap
  [128,32] reciprocal -> transposed back -> rank-8 indicator matmul
  broadcast (replaces 3.3us-per-call wide DVE reciprocals).
- x streamed per 512-column block (2-deep) instead of fully resident.
- yproj results DMA'd directly from PSUM to DRAM.
"""
import sys
import os
import math
from contextlib import ExitStack

sys.path.insert(0, "/opt/trn_rl_repo")

import numpy as np
import ml_dtypes

BF16 = ml_dtypes.bfloat16

B, S, DIM = 2, 2048, 2048
HEADS, DH = 32, 64
THETA = 10000.0
NCORES = 8
TP = 4             # head-parallel ways
HPC = HEADS // TP  # heads per core = 8
E = HPC * DH       # per-core qkv width = 512
ET = E // 128      # e-tiles per core = 4
DT = DIM // 128    # contraction d-tiles = 16
SB = S // 512      # 512-wide seq blocks = 4
SS = S // 128      # 128-wide seq blocks = 16

_CACHE = {}


def _l2n(w, axis):
    n = np.sqrt((w.astype(np.float64) ** 2).sum(axis=axis, keepdims=True))
    n = np.maximum(n, 1e-12)
    return (w / n).astype(np.float32)


def _build_program():
    import concourse.bass as bass
    from concourse import bacc
    import concourse.mybir as mybir
    import concourse.tile as tile
    from concourse.masks import make_identity

    f32 = mybir.dt.float32
    bf16 = mybir.dt.bfloat16
    AF = mybir.ActivationFunctionType
    AX = mybir.AxisListType
    OP = mybir.AluOpType

    nc = bacc.Bacc("TRN2", target_bir_lowering=False)

    xT = nc.dram_tensor("xT", [DIM, S], bf16, kind="ExternalInput")
    wqT = nc.dram_tensor("wqT", [DIM, E], bf16, kind="ExternalInput")
    wkT = nc.dram_tensor("wkT", [DIM, E], bf16, kind="ExternalInput")
    wvT = nc.dram_tensor("wvT", [DIM, E], bf16, kind="ExternalInput")
    woT = nc.dram_tensor("woT", [E, DIM], bf16, kind="ExternalInput")
    cosd = nc.dram_tensor("cosd", [128, SS * DH], bf16, kind="ExternalInput")
    sind = nc.dram_tensor("sind", [128, SS * DH], bf16, kind="ExternalInput")
    trid = nc.dram_tensor("trid", [128, 128], bf16, kind="ExternalInput")
    ind8d = nc.dram_tensor("ind8d", [8, 512], bf16, kind="ExternalInput")
    Y = nc.dram_tensor("Y", [S, DIM], f32, kind="ExternalOutput")

    with tile.TileContext(nc) as tc, ExitStack() as ctx:
        const = ctx.enter_context(tc.tile_pool(name="const", bufs=1))
        wpool = ctx.enter_context(tc.tile_pool(name="wpool", bufs=4))
        xpool = ctx.enter_context(tc.tile_pool(name="xpool", bufs=2))
        qkv = ctx.enter_context(tc.tile_pool(name="qkv", bufs=1))
        work = ctx.enter_context(tc.tile_pool(name="work", bufs=1))
        expool = ctx.enter_context(tc.tile_pool(name="expool", bufs=4))
        psA = ctx.enter_context(
            tc.tile_pool(name="psA", bufs=4, space="PSUM"))
        psL = ctx.enter_context(
            tc.tile_pool(name="psL", bufs=2, space="PSUM"))

        # --- weights (wq first, quartered, so proj can start early) ---
        wq_sb = [wpool.tile([128, 4, E], bf16, tag=f"wq{j}", bufs=1, name=f"wq{j}")
                 for j in range(4)]
        wk_sb = wpool.tile([128, DT, E], bf16, tag="wk", bufs=1)
        wv_sb = wpool.tile([128, DT, E], bf16, tag="wv", bufs=1)
        wo_sb = wpool.tile([128, ET, DIM], bf16, tag="wo", bufs=1)
        wqr = wqT.rearrange("(t p) e -> p t e", p=128)

        xtiles = {}

        def load_x(st):
            ts = [xpool.tile([128, 4, 512], bf16, tag=f"x{j}", bufs=2,
                             name=f"xst{st}_{j}") for j in range(4)]
            src = xT[:, st * 512:(st + 1) * 512].rearrange(
                "(t p) s -> p t s", p=128)
            for j in range(4):
                nc.sync.dma_start(ts[j], src[:, j * 4:(j + 1) * 4, :])
            return ts

        # interleave wq quarters with x quarters so dt=0..3 can start early
        x0src = xT[:, 0:512].rearrange("(t p) s -> p t s", p=128)
        x0 = [xpool.tile([128, 4, 512], bf16, tag=f"x{j}", bufs=2,
                         name=f"xst0_{j}") for j in range(4)]
        for j in range(4):
            nc.sync.dma_start(wq_sb[j], wqr[:, j * 4:(j + 1) * 4, :])
            nc.sync.dma_start(x0[j], x0src[:, j * 4:(j + 1) * 4, :])
        xtiles[0] = x0
        nc.sync.dma_start(wk_sb, wkT.rearrange("(t p) e -> p t e", p=128))
        nc.sync.dma_start(wv_sb, wvT.rearrange("(t p) e -> p t e", p=128))

        # --- constants ---
        cos_sb = const.tile([128, SS, DH], bf16)
        sin_sb = const.tile([128, SS, DH], bf16)
        nc.sync.dma_start(cos_sb, cosd.rearrange("p (b d) -> p b d", d=DH))
        nc.sync.dma_start(sin_sb, sind.rearrange("p (b d) -> p b d", d=DH))
        tri = const.tile([128, 128], bf16)
        nc.sync.dma_start(tri, trid[:, :])
        ind8 = const.tile([8, 512], bf16)
        nc.sync.dma_start(ind8, ind8d[:, :])
        nc.sync.dma_start(wo_sb, woT.rearrange("(t p) e -> p t e", p=128))
        identf = const.tile([128, 128], f32)
        make_identity(nc, identf)
        ident = const.tile([128, 128], bf16)
        make_identity(nc, ident)

        # --- persistent activations ---
        qTall = qkv.tile([128, ET, S], bf16, tag="qT")
        kTall = qkv.tile([128, ET, S], bf16, tag="kT")
        v_sb = qkv.tile([128, SS, HPC, 66], bf16, tag="v")
        stash = qkv.tile([65, HPC, 512], bf16, tag="stash")
        nc.vector.memset(v_sb[:, :, :, 64:66], 1.0)

        def norm_rope(ps, dstT, st, su):
            """psum [si,e] natural -> per-head l2norm, rope, bf16,
            -> DMA-transpose into dstT columns."""
            sblk = st * 4 + su
            sq = work.tile([128, E], bf16, tag="sq", bufs=2)
            nc.scalar.square(sq, ps)
            ssq = work.tile([128, HPC], f32, tag="ssq", bufs=2)
            nc.vector.tensor_reduce(
                ssq, sq.rearrange("p (h d) -> p h d", d=DH),
                axis=AX.X, op=OP.add)
            nc.scalar.sqrt(ssq, ssq)
            inv = work.tile([128, HPC], f32, tag="inv", bufs=2)
            nc.vector.reciprocal(inv, ssq)
            qn = work.tile([128, HPC, DH], bf16, tag="qn", bufs=2)
            nc.vector.tensor_mul(
                qn, ps.rearrange("p (h d) -> p h d", d=DH),
                inv.unsqueeze(2).broadcast_to([128, HPC, DH]))
            cosb = cos_sb[:, sblk:sblk + 1, :].broadcast_to([128, HPC, DH])
            sinb = sin_sb[:, sblk:sblk + 1, :].broadcast_to([128, HPC, DH])
            rot = work.tile([128, HPC, 2, 32], bf16, tag="rot", bufs=2)
            qn4 = qn.rearrange("p h (t u) -> p h t u", u=32)
            nc.vector.tensor_copy(rot[:, :, 0:1, :], qn4[:, :, 1:2, :])
            nc.vector.tensor_copy(rot[:, :, 1:2, :], qn4[:, :, 0:1, :])
            nc.vector.tensor_mul(rot.rearrange("p h t u -> p h (t u)"),
                                 rot.rearrange("p h t u -> p h (t u)"), sinb)
            nc.vector.tensor_mul(qn, qn, cosb)
            qo = work.tile([128, E], bf16, tag="qo", bufs=2)
            nc.vector.tensor_add(
                qo, qn.rearrange("p h d -> p (h d)"),
                rot.rearrange("p h t u -> p (h t u)"))
            nc.sync.dma_start_transpose(
                dstT[:, :, sblk * 128:(sblk + 1) * 128], qo)

        def proj_wave(w_sb, kind, st, xt):
            quartered = isinstance(w_sb, list)
            for s0 in (0, 2):
                prs = [psA.tile([128, E], f32, tag="ps",
                                name=f"p{kind}{st}_{s0 + j}")
                       for j in range(2)]
                for dt in range(DT):
                    if quartered:
                        wslice = w_sb[dt // 4][:, dt % 4, :]
                    else:
                        wslice = w_sb[:, dt, :]
                    for j in range(2):
                        su = s0 + j
                        nc.tensor.matmul(
                            prs[j],
                            xt[dt // 4][:, dt % 4,
                                        su * 128:(su + 1) * 128],
                            wslice,
                            start=(dt == 0), stop=(dt == DT - 1))
                for j in range(2):
                    su = s0 + j
                    if kind == "v":
                        nc.vector.tensor_copy(
                            v_sb[:, st * 4 + su, :, 0:64],
                            prs[j].rearrange("p (h d) -> p h d", d=DH))
                    else:
                        norm_rope(prs[j], qTall if kind == "q" else kTall,
                                  st, su)

        def proj_all(st):
            xt = xtiles[st]
            proj_wave(wq_sb, "q", st, xt)
            proj_wave(wk_sb, "k", st, xt)
            proj_wave(wv_sb, "v", st, xt)

        def attn_block(i):
            last = 4 * i + 3
            for h in range(HPC):
                et, hp = h // 2, (h % 2) * 64
                pv = psA.tile([128, 512], f32, tag="ps", name=f"pv{i}_{h}")
                npr = 2 * (i + 1)
                lgs = {}

                def emit_lg_b(p, b):
                    if p not in lgs:
                        lgs[p] = psL.tile([128, 2, 512], f32, tag="lg",
                                          name=f"lg{i}_{h}_{p}")
                    sjb = 2 * p + b
                    r = sjb - 4 * i
                    c0 = r * 128 if r > 0 else 0
                    nc.tensor.matmul(
                        lgs[p][:, b, c0:],
                        kTall[hp:hp + 64, et, sjb * 128:(sjb + 1) * 128],
                        qTall[hp:hp + 64, et, i * 512 + c0:(i + 1) * 512],
                        start=True, stop=True)

                def emit_lg(p):
                    emit_lg_b(p, 0)
                    emit_lg_b(p, 1)

                emit_lg(0)
                if npr > 1:
                    emit_lg(1)
                for p in range(npr):
                    lg2 = lgs.pop(p)
                    ex = expool.tile([128, 2, 512], bf16, tag="ex")
                    if 2 * p - 4 * i >= 0:  # diagonal pair: match lg trim
                        for b in range(2):
                            c0 = max(0, (2 * p + b - 4 * i)) * 128
                            nc.scalar.activation(ex[:, b, c0:],
                                                 lg2[:, b, c0:], AF.Exp)
                    else:
                        nc.scalar.activation(ex, lg2, AF.Exp)
                    for b in range(2):
                        sjb = 2 * p + b
                        r = sjb - 4 * i
                        if r >= 0:
                            nc.gpsimd.tensor_mul(
                                ex[:, b, r * 128:(r + 1) * 128],
                                ex[:, b, r * 128:(r + 1) * 128], tri)
                        c0 = r * 128 if r > 0 else 0
                        nc.tensor.matmul(
                            pv[0:66, c0:],
                            v_sb[:, sjb, h, :],
                            ex[:, b, c0:],
                            start=(sjb == 0), stop=(sjb == last))
                        if p + 2 < npr:
                            emit_lg_b(p + 2, b)
                nc.vector.tensor_copy(stash[:, h, :], pv[0:65, :])

        def normalize_gather(i):
            den = work.tile([8, 512], bf16, tag="den", bufs=2)
            nc.scalar.dma_start(den, stash[64:65, :, :])
            return den

        def normalize_recip(i, den):
            """den rows -> si-partition layout via PE transposes -> one
            cheap [128,32] DVE reciprocal."""
            invT = psA.tile([128, 32], bf16, tag="ps")
            for c in range(4):
                nc.tensor.transpose(
                    invT[:, c * 8:(c + 1) * 8],
                    den[:, c * 128:(c + 1) * 128], ident[0:8, 0:8])
            inv_sb = work.tile([128, 32], f32, tag="invsb", bufs=2)
            nc.vector.reciprocal(inv_sb, invT)
            return inv_sb

        def normalize_apply(i, inv_sb):
            """transpose back to row layout, rank-8 indicator broadcast,
            per-head mul into qTall."""
            invrow = psA.tile([8, 4, 128], f32, tag="ps")
            for c in range(4):
                nc.tensor.transpose(
                    invrow[:, c, :], inv_sb[:, c * 8:(c + 1) * 8], identf)
            inv_row = work.tile([8, 512], bf16, tag="invrowsb", bufs=2)
            nc.vector.tensor_copy(
                inv_row, invrow.rearrange("p c j -> p (c j)"))
            for h in range(HPC):
                et, hp = h // 2, (h % 2) * 64
                bc = psA.tile([64, 512], f32, tag="ps", name=f"bc{i}_{h}")
                nc.tensor.matmul(bc, ind8[:, h * 64:(h + 1) * 64], inv_row,
                                 start=True, stop=True)
                nc.vector.tensor_mul(
                    qTall[hp:hp + 64, et, i * 512:(i + 1) * 512],
                    stash[0:64, h, :], bc)

        def yproj_block(i):
            for ib in range(4 * i, 4 * i + 4):
                for nd0 in (0, 2):
                    pss = [psA.tile([128, 512], f32, tag="ps",
                                    name=f"y{ib}_{nd0 + j}")
                           for j in range(2)]
                    for ket in range(ET):
                        for j in range(2):
                            nd = nd0 + j
                            nc.tensor.matmul(
                                pss[j],
                                qTall[:, ket, ib * 128:(ib + 1) * 128],
                                wo_sb[:, ket, nd * 512:(nd + 1) * 512],
                                start=(ket == 0), stop=(ket == ET - 1))
                    for j in range(2):
                        nd = nd0 + j
                        ys = work.tile([128, 512], f32, tag="ys", bufs=2)
                        if nd % 2 == 0:
                            nc.vector.tensor_copy(ys, pss[j])
                        else:
                            nc.scalar.copy(ys, pss[j])
                        nc.sync.dma_start(
                            Y[ib * 128:(ib + 1) * 128,
                              nd * 512:(nd + 1) * 512], ys)

        proj_all(0)
        for st in range(SB):
            if st + 1 < SB:
                xtiles[st + 1] = load_x(st + 1)
            attn_block(st)
            den = normalize_gather(st)
            inv_sb = normalize_recip(st, den)
            if st + 1 < SB:
                proj_wave(wq_sb, "q", st + 1, xtiles[st + 1])
                normalize_apply(st, inv_sb)
                proj_wave(wk_sb, "k", st + 1, xtiles[st + 1])
                proj_wave(wv_sb, "v", st + 1, xtiles[st + 1])
            else:
                normalize_apply(st, inv_sb)
            yproj_block(st)

    return nc


def _host_prep(x, wq, wk, wv, wo, qk_scale):
    """Returns per-core input dicts."""
    perm = np.concatenate([np.arange(0, DH, 2), np.arange(1, DH, 2)])
    wq_n = _l2n(wq, -1).reshape(HEADS, DH, DIM)[:, perm, :].reshape(HEADS * DH, DIM)
    wk_n = _l2n(wk, -1).reshape(HEADS, DH, DIM)[:, perm, :].reshape(HEADS * DH, DIM)
    wv_n = _l2n(wv, -1)
    wo_n = _l2n(wo, 0)
    sp = qk_scale.astype(np.float64)[perm]

    # rope tables with qk_scale folded in; permuted-block layout
    half = np.arange(0, DH, 2)
    freqs = 1.0 / (THETA ** (half.astype(np.float64) / DH))      # (32,)
    ang = np.arange(S, dtype=np.float64)[:, None] * freqs[None]  # (S, 32)
    cos_h, sin_h = np.cos(ang), np.sin(ang)
    cos_p = np.concatenate([cos_h, cos_h], 1)                    # (S, 64)
    sin_e = np.concatenate([-sin_h, sin_h], 1)
    cos_eff = (cos_p * sp[None, :]).astype(np.float32)
    swap_sp = np.concatenate([sp[32:], sp[:32]])
    sin_eff = (sin_e * swap_sp[None, :]).astype(np.float32)
    # device layout [128, SS*DH]: [p, b*64+c] = tbl[b*128+p, c]
    cosd = np.ascontiguousarray(
        cos_eff.reshape(SS, 128, DH).transpose(1, 0, 2).reshape(128, SS * DH))
    sind = np.ascontiguousarray(
        sin_eff.reshape(SS, 128, DH).transpose(1, 0, 2).reshape(128, SS * DH))

    # causal triangle for the diagonal 128-blocks: keep sjl <= sil
    sjl = np.arange(128)[:, None]
    sil = np.arange(128)[None, :]
    trid = (sjl <= sil).astype(np.float32)

    # indicator for denominator broadcast: ind8[k, h*64+m] = (k == h)
    ind8 = np.zeros((8, 512), dtype=np.float32)
    for h in range(8):
        ind8[h, h * 64:(h + 1) * 64] = 1.0

    in_maps = []
    for c in range(NCORES):
        b, t = divmod(c, TP)
        e0 = t * E
        in_maps.append({
            "xT": np.ascontiguousarray(x[b].T).astype(BF16),
            "wqT": np.ascontiguousarray(wq_n[e0:e0 + E].T).astype(BF16),
            "wkT": np.ascontiguousarray(wk_n[e0:e0 + E].T).astype(BF16),
            "wvT": np.ascontiguousarray(wv_n[e0:e0 + E].T).astype(BF16),
            "woT": np.ascontiguousarray(wo_n[:, e0:e0 + E].T).astype(BF16),
            "cosd": cosd.astype(BF16), "sind": sind.astype(BF16),
            "trid": trid.astype(BF16), "ind8d": ind8.astype(BF16),
        })
    return in_maps


def _install_profile_hook():
    """antenv.axon_hooks is absent in this image; shim it and register the
    ctypes NTFF hook against /opt/axon/libaxon_pjrt.so (mirrors trn_boot)."""
    import types
    import ctypes
    import contextlib

    try:
        from antenv.axon_hooks import get_axon_ntff_profile_hook  # noqa
        return
    except ImportError:
        pass
    import antenv
    mod = types.ModuleType("antenv.axon_hooks")
    state = {}
    mod.set_axon_ntff_profile_hook = lambda h: state.__setitem__("h", h)
    mod.get_axon_ntff_profile_hook = lambda: state.get("h")
    sys.modules["antenv.axon_hooks"] = mod
    antenv.axon_hooks = mod

    so_path = "/opt/axon/libaxon_pjrt.so"
    lib = ctypes.CDLL(so_path)
    if not hasattr(lib, "axon_start_nrt_profile"):
        return
    lib.axon_start_nrt_profile.argtypes = [
        ctypes.POINTER(ctypes.c_int64), ctypes.c_size_t]
    lib.axon_start_nrt_profile.restype = ctypes.c_int64
    lib.axon_stop_nrt_profile.argtypes = [ctypes.c_char_p]
    lib.axon_stop_nrt_profile.restype = ctypes.c_int64

    @contextlib.contextmanager
    def _hook(output_dir, device_ids):
        import jax
        jax.devices()
        if device_ids:
            ids = (ctypes.c_int64 * len(device_ids))(*device_ids)
            rc = lib.axon_start_nrt_profile(ids, len(device_ids))
        else:
            rc = lib.axon_start_nrt_profile(None, 0)
        if rc != 0:
            raise RuntimeError(f"axon_start_nrt_profile rc={rc}")
        try:
            yield
        finally:
            n = lib.axon_stop_nrt_profile(str(output_dir).encode())
            print(f"profile: {n} file(s) written to {output_dir}",
                  file=sys.stderr)

    mod.set_axon_ntff_profile_hook(_hook)


def kernel(x, wq, wk, wv, wo, qk_scale, _profile=False):
    from concourse.bass_utils import run_bass_kernel_spmd

    if _profile:
        _install_profile_hook()

    if "nc" not in _CACHE:
        nc = _build_program()
        nc.finalize()
        _CACHE["nc"] = nc
    nc = _CACHE["nc"]
    in_maps = _host_prep(np.asarray(x), np.asarray(wq), np.asarray(wk),
                         np.asarray(wv), np.asarray(wo), np.asarray(qk_scale))
    res = run_bass_kernel_spmd(nc, in_maps, core_ids=list(range(NCORES)),
                               trace=_profile)
    outs = res.results
    y = np.empty((B, S, DIM), dtype=np.float32)
    for b in range(B):
        y[b] = sum(outs[b * TP + t]["Y"] for t in range(TP))
    if _profile:
        _CACHE["last_exec_time_ns"] = res.exec_time_ns
        _CACHE["last_profile"] = res.profile_json
    return y


# revision 19
# speedup vs baseline: 1.1013x; 1.0062x over previous
"""nn_Attention Trainium2 Bass kernel (v2 — interleaved pipeline).

Full attention forward: x->(q,k,v) with l2-normalized weights, per-head-dim
l2 norm + learned qk scale, interleaved RoPE, causal SDPA, output projection
with column-l2-normalized wo.

Sharding: TP=4 over heads (8 heads/core) x DP=2 over batch across 8 cores.
Each core computes a partial [2048, 2048] output for its batch; host sums
the 4 TP partials per batch.

v2 changes vs v1:
- single interleaved loop per 512-row block: proj -> attention -> yproj,
  so DVE rope work, Act exp work and PE matmuls overlap across phases.
- q/k transposes via DMA xbar (dma_start_transpose) instead of PE
  transposes + DVE copies.
- causal mask as a single 128x128 triangle multiply on the Pool engine.
- lg/pv matmuls trimmed to the live columns on diagonal blocks.
- softmax denominators: v's 65th ones-column -> psum row 64 -> stashed ->
  gathered by DMA -> PE-transposed to si-partition layout -> one cheap
  [128,32] reciprocal -> transposed back -> rank-8 indicator matmul
  broadcast (replaces 3.3us-per-call wide DVE reciprocals).
- x streamed per 512-column block (2-deep) instead of fully resident.
- yproj results DMA'd directly from PSUM to DRAM.
"""
import sys
import os
import math
from contextlib import ExitStack

sys.path.insert(0, "/opt/trn_rl_repo")

import numpy as np
import ml_dtypes

BF16 = ml_dtypes.bfloat16

B, S, DIM = 2, 2048, 2048
HEADS, DH = 32, 64
THETA = 10000.0
NCORES = 8
TP = 4             # head-parallel ways
HPC = HEADS // TP  # heads per core = 8
E = HPC * DH       # per-core qkv width = 512
ET = E // 128      # e-tiles per core = 4
DT = DIM // 128    # contraction d-tiles = 16
SB = S // 512      # 512-wide seq blocks = 4
SS = S // 128      # 128-wide seq blocks = 16

_CACHE = {}


def _l2n(w, axis):
    n = np.sqrt((w.astype(np.float64) ** 2).sum(axis=axis, keepdims=True))
    n = np.maximum(n, 1e-12)
    return (w / n).astype(np.float32)


def _build_program():
    import concourse.bass as bass
    from concourse import bacc
    import concourse.mybir as mybir
    import concourse.tile as tile
    from concourse.masks import make_identity

    f32 = mybir.dt.float32
    bf16 = mybir.dt.bfloat16
    AF = mybir.ActivationFunctionType
    AX = mybir.AxisListType
    OP = mybir.AluOpType

    nc = bacc.Bacc("TRN2", target_bir_lowering=False)

    xT = nc.dram_tensor("xT", [DIM, S], bf16, kind="ExternalInput")
    wqT = nc.dram_tensor("wqT", [DIM, E], bf16, kind="ExternalInput")
    wkT = nc.dram_tensor("wkT", [DIM, E], bf16, kind="ExternalInput")
    wvT = nc.dram_tensor("wvT", [DIM, E], bf16, kind="ExternalInput")
    woT = nc.dram_tensor("woT", [E, DIM], bf16, kind="ExternalInput")
    cosd = nc.dram_tensor("cosd", [128, SS * DH], bf16, kind="ExternalInput")
    sind = nc.dram_tensor("sind", [128, SS * DH], bf16, kind="ExternalInput")
    trid = nc.dram_tensor("trid", [128, 128], bf16, kind="ExternalInput")
    ind8d = nc.dram_tensor("ind8d", [8, 512], bf16, kind="ExternalInput")
    Y = nc.dram_tensor("Y", [S, DIM], f32, kind="ExternalOutput")

    with tile.TileContext(nc) as tc, ExitStack() as ctx:
        const = ctx.enter_context(tc.tile_pool(name="const", bufs=1))
        wpool = ctx.enter_context(tc.tile_pool(name="wpool", bufs=4))
        xpool = ctx.enter_context(tc.tile_pool(name="xpool", bufs=2))
        qkv = ctx.enter_context(tc.tile_pool(name="qkv", bufs=1))
        work = ctx.enter_context(tc.tile_pool(name="work", bufs=1))
        expool = ctx.enter_context(tc.tile_pool(name="expool", bufs=4))
        psA = ctx.enter_context(
            tc.tile_pool(name="psA", bufs=4, space="PSUM"))
        psL = ctx.enter_context(
            tc.tile_pool(name="psL", bufs=2, space="PSUM"))

        # --- weights (wq first, quartered, so proj can start early) ---
        wq_sb = [wpool.tile([128, 4, E], bf16, tag=f"wq{j}", bufs=1, name=f"wq{j}")
                 for j in range(4)]
        wk_sb = wpool.tile([128, DT, E], bf16, tag="wk", bufs=1)
        wv_sb = wpool.tile([128, DT, E], bf16, tag="wv", bufs=1)
        wo_sb = wpool.tile([128, ET, DIM], bf16, tag="wo", bufs=1)
        wqr = wqT.rearrange("(t p) e -> p t e", p=128)

        xtiles = {}

        def load_x(st):
            ts = [xpool.tile([128, 4, 512], bf16, tag=f"x{j}", bufs=2,
                             name=f"xst{st}_{j}") for j in range(4)]
            src = xT[:, st * 512:(st + 1) * 512].rearrange(
                "(t p) s -> p t s", p=128)
            for j in range(4):
                nc.sync.dma_start(ts[j], src[:, j * 4:(j + 1) * 4, :])
            return ts

        # interleave wq quarters with x quarters so dt=0..3 can start early
        x0src = xT[:, 0:512].rearrange("(t p) s -> p t s", p=128)
        x0 = [xpool.tile([128, 4, 512], bf16, tag=f"x{j}", bufs=2,
                         name=f"xst0_{j}") for j in range(4)]
        for j in range(4):
            nc.sync.dma_start(wq_sb[j], wqr[:, j * 4:(j + 1) * 4, :])
            nc.sync.dma_start(x0[j], x0src[:, j * 4:(j + 1) * 4, :])
        xtiles[0] = x0
        nc.sync.dma_start(wk_sb, wkT.rearrange("(t p) e -> p t e", p=128))
        nc.sync.dma_start(wv_sb, wvT.rearrange("(t p) e -> p t e", p=128))

        # --- constants ---
        cos_sb = const.tile([128, SS, DH], bf16)
        sin_sb = const.tile([128, SS, DH], bf16)
        nc.sync.dma_start(cos_sb, cosd.rearrange("p (b d) -> p b d", d=DH))
        nc.sync.dma_start(sin_sb, sind.rearrange("p (b d) -> p b d", d=DH))
        tri = const.tile([128, 128], bf16)
        nc.sync.dma_start(tri, trid[:, :])
        ind8 = const.tile([8, 512], bf16)
        nc.sync.dma_start(ind8, ind8d[:, :])
        nc.sync.dma_start(wo_sb, woT.rearrange("(t p) e -> p t e", p=128))
        identf = const.tile([128, 128], f32)
        make_identity(nc, identf)
        ident = const.tile([128, 128], bf16)
        make_identity(nc, ident)

        # --- persistent activations ---
        qTall = qkv.tile([128, ET, S], bf16, tag="qT")
        kTall = qkv.tile([128, ET, S], bf16, tag="kT")
        v_sb = qkv.tile([128, SS, HPC, 66], bf16, tag="v")
        stash = qkv.tile([65, HPC, 512], bf16, tag="stash")
        nc.vector.memset(v_sb[:, :, :, 64:66], 1.0)

        def norm_rope(ps, dstT, st, su):
            """psum [si,e] natural -> per-head l2norm, rope, bf16,
            -> DMA-transpose into dstT columns."""
            sblk = st * 4 + su
            sq = work.tile([128, E], bf16, tag="sq", bufs=2)
            nc.scalar.square(sq, ps)
            ssq = work.tile([128, HPC], f32, tag="ssq", bufs=2)
            nc.vector.tensor_reduce(
                ssq, sq.rearrange("p (h d) -> p h d", d=DH),
                axis=AX.X, op=OP.add)
            nc.scalar.sqrt(ssq, ssq)
            inv = work.tile([128, HPC], f32, tag="inv", bufs=2)
            nc.vector.reciprocal(inv, ssq)
            qn = work.tile([128, HPC, DH], bf16, tag="qn", bufs=2)
            nc.vector.tensor_mul(
                qn, ps.rearrange("p (h d) -> p h d", d=DH),
                inv.unsqueeze(2).broadcast_to([128, HPC, DH]))
            cosb = cos_sb[:, sblk:sblk + 1, :].broadcast_to([128, HPC, DH])
            sinb = sin_sb[:, sblk:sblk + 1, :].broadcast_to([128, HPC, DH])
            rot = work.tile([128, HPC, 2, 32], bf16, tag="rot", bufs=2)
            qn4 = qn.rearrange("p h (t u) -> p h t u", u=32)
            nc.vector.tensor_copy(rot[:, :, 0:1, :], qn4[:, :, 1:2, :])
            nc.vector.tensor_copy(rot[:, :, 1:2, :], qn4[:, :, 0:1, :])
            nc.vector.tensor_mul(rot.rearrange("p h t u -> p h (t u)"),
                                 rot.rearrange("p h t u -> p h (t u)"), sinb)
            nc.vector.tensor_mul(qn, qn, cosb)
            qo = work.tile([128, E], bf16, tag="qo", bufs=2)
            nc.vector.tensor_add(
                qo, qn.rearrange("p h d -> p (h d)"),
                rot.rearrange("p h t u -> p (h t u)"))
            nc.sync.dma_start_transpose(
                dstT[:, :, sblk * 128:(sblk + 1) * 128], qo)

        def proj_wave(w_sb, kind, st, xt):
            quartered = isinstance(w_sb, list)
            for s0 in (0, 2):
                prs = [psA.tile([128, E], f32, tag="ps",
                                name=f"p{kind}{st}_{s0 + j}")
                       for j in range(2)]
                for dt in range(DT):
                    if quartered:
                        wslice = w_sb[dt // 4][:, dt % 4, :]
                    else:
                        wslice = w_sb[:, dt, :]
                    for j in range(2):
                        su = s0 + j
                        nc.tensor.matmul(
                            prs[j],
                            xt[dt // 4][:, dt % 4,
                                        su * 128:(su + 1) * 128],
                            wslice,
                            start=(dt == 0), stop=(dt == DT - 1))
                for j in range(2):
                    su = s0 + j
                    if kind == "v":
                        nc.vector.tensor_copy(
                            v_sb[:, st * 4 + su, :, 0:64],
                            prs[j].rearrange("p (h d) -> p h d", d=DH))
                    else:
                        norm_rope(prs[j], qTall if kind == "q" else kTall,
                                  st, su)

        def proj_all(st):
            xt = xtiles[st]
            proj_wave(wq_sb, "q", st, xt)
            proj_wave(wk_sb, "k", st, xt)
            proj_wave(wv_sb, "v", st, xt)

        def attn_block(i):
            """Head-paired attention: heads (h, h+2) share PE tile config
            (same hp), so lg and pv matmuls run in same-config groups of 4
            with alternating PSUM banks."""
            last = 4 * i + 3
            npr = 2 * (i + 1)
            for ha, hb in ((0, 2), (4, 6), (1, 3), (5, 7)):
                hp = (ha % 2) * 64
                ets = {ha: ha // 2, hb: hb // 2}
                pvs = {h: psA.tile([128, 512], f32, tag="ps",
                                   name=f"pv{i}_{h}")
                       for h in (ha, hb)}
                lgs = {}

                def lg4(p):
                    for h in (ha, hb):
                        lgs[(h, p)] = psL.tile(
                            [128, 2, 512], f32, tag="lg",
                            name=f"lg{i}_{h}_{p}")
                    for b in range(2):
                        sjb = 2 * p + b
                        r = sjb - 4 * i
                        c0 = r * 128 if r > 0 else 0
                        for h in (ha, hb):
                            nc.tensor.matmul(
                                lgs[(h, p)][:, b, c0:],
                                kTall[hp:hp + 64, ets[h],
                                      sjb * 128:(sjb + 1) * 128],
                                qTall[hp:hp + 64, ets[h],
                                      i * 512 + c0:(i + 1) * 512],
                                start=True, stop=True)

                lg4(0)
                for p in range(npr):
                    exs = {}
                    for h in (ha, hb):
                        lg2 = lgs.pop((h, p))
                        ex = expool.tile([128, 2, 512], bf16, tag="ex",
                                         name=f"ex{i}_{h}_{p}")
                        if 2 * p - 4 * i >= 0:  # diagonal pair: match trim
                            for b in range(2):
                                c0 = max(0, (2 * p + b - 4 * i)) * 128
                                nc.scalar.activation(ex[:, b, c0:],
                                                     lg2[:, b, c0:], AF.Exp)
                        else:
                            nc.scalar.activation(ex, lg2, AF.Exp)
                        exs[h] = ex
                    if p + 1 < npr:
                        lg4(p + 1)
                    for b in range(2):
                        sjb = 2 * p + b
                        r = sjb - 4 * i
                        c0 = r * 128 if r > 0 else 0
                        if r >= 0:
                            for h in (ha, hb):
                                nc.gpsimd.tensor_mul(
                                    exs[h][:, b, r * 128:(r + 1) * 128],
                                    exs[h][:, b, r * 128:(r + 1) * 128],
                                    tri)
                        for h in (ha, hb):
                            nc.tensor.matmul(
                                pvs[h][0:66, c0:],
                                v_sb[:, sjb, h, :],
                                exs[h][:, b, c0:],
                                start=(sjb == 0), stop=(sjb == last))
                for h in (ha, hb):
                    nc.vector.tensor_copy(stash[:, h, :], pvs[h][0:65, :])

        def normalize_gather(i):
            den = work.tile([8, 512], bf16, tag="den", bufs=2)
            nc.scalar.dma_start(den, stash[64:65, :, :])
            return den

        def normalize_recip(i, den):
            """den rows -> si-partition layout via PE transposes -> one
            cheap [128,32] DVE reciprocal."""
            invT = psA.tile([128, 32], bf16, tag="ps")
            for c in range(4):
                nc.tensor.transpose(
                    invT[:, c * 8:(c + 1) * 8],
                    den[:, c * 128:(c + 1) * 128], ident[0:8, 0:8])
            inv_sb = work.tile([128, 32], f32, tag="invsb", bufs=2)
            nc.vector.reciprocal(inv_sb, invT)
            return inv_sb

        def normalize_apply(i, inv_sb):
            """transpose back to row layout, rank-8 indicator broadcast,
            per-head mul into qTall."""
            invrow = psA.tile([8, 4, 128], f32, tag="ps")
            for c in range(4):
                nc.tensor.transpose(
                    invrow[:, c, :], inv_sb[:, c * 8:(c + 1) * 8], identf)
            inv_row = work.tile([8, 512], bf16, tag="invrowsb", bufs=2)
            nc.vector.tensor_copy(
                inv_row, invrow.rearrange("p c j -> p (c j)"))
            for h in range(HPC):
                et, hp = h // 2, (h % 2) * 64
                bc = psA.tile([64, 512], f32, tag="ps", name=f"bc{i}_{h}")
                nc.tensor.matmul(bc, ind8[:, h * 64:(h + 1) * 64], inv_row,
                                 start=True, stop=True)
                nc.vector.tensor_mul(
                    qTall[hp:hp + 64, et, i * 512:(i + 1) * 512],
                    stash[0:64, h, :], bc)

        def yproj_block(i):
            for ib in range(4 * i, 4 * i + 4):
                for nd0 in (0, 2):
                    pss = [psA.tile([128, 512], f32, tag="ps",
                                    name=f"y{ib}_{nd0 + j}")
                           for j in range(2)]
                    for ket in range(ET):
                        for j in range(2):
                            nd = nd0 + j
                            nc.tensor.matmul(
                                pss[j],
                                qTall[:, ket, ib * 128:(ib + 1) * 128],
                                wo_sb[:, ket, nd * 512:(nd + 1) * 512],
                                start=(ket == 0), stop=(ket == ET - 1))
                    for j in range(2):
                        nd = nd0 + j
                        ys = work.tile([128, 512], f32, tag="ys", bufs=2)
                        if nd % 2 == 0:
                            nc.vector.tensor_copy(ys, pss[j])
                        else:
                            nc.scalar.copy(ys, pss[j])
                        nc.sync.dma_start(
                            Y[ib * 128:(ib + 1) * 128,
                              nd * 512:(nd + 1) * 512], ys)

        proj_all(0)
        for st in range(SB):
            if st + 1 < SB:
                xtiles[st + 1] = load_x(st + 1)
            attn_block(st)
            den = normalize_gather(st)
            inv_sb = normalize_recip(st, den)
            if st + 1 < SB:
                proj_wave(wq_sb, "q", st + 1, xtiles[st + 1])
                normalize_apply(st, inv_sb)
                proj_wave(wk_sb, "k", st + 1, xtiles[st + 1])
                proj_wave(wv_sb, "v", st + 1, xtiles[st + 1])
            else:
                normalize_apply(st, inv_sb)
            yproj_block(st)

    return nc


def _host_prep(x, wq, wk, wv, wo, qk_scale):
    """Returns per-core input dicts."""
    perm = np.concatenate([np.arange(0, DH, 2), np.arange(1, DH, 2)])
    wq_n = _l2n(wq, -1).reshape(HEADS, DH, DIM)[:, perm, :].reshape(HEADS * DH, DIM)
    wk_n = _l2n(wk, -1).reshape(HEADS, DH, DIM)[:, perm, :].reshape(HEADS * DH, DIM)
    wv_n = _l2n(wv, -1)
    wo_n = _l2n(wo, 0)
    sp = qk_scale.astype(np.float64)[perm]

    # rope tables with qk_scale folded in; permuted-block layout
    half = np.arange(0, DH, 2)
    freqs = 1.0 / (THETA ** (half.astype(np.float64) / DH))      # (32,)
    ang = np.arange(S, dtype=np.float64)[:, None] * freqs[None]  # (S, 32)
    cos_h, sin_h = np.cos(ang), np.sin(ang)
    cos_p = np.concatenate([cos_h, cos_h], 1)                    # (S, 64)
    sin_e = np.concatenate([-sin_h, sin_h], 1)
    cos_eff = (cos_p * sp[None, :]).astype(np.float32)
    swap_sp = np.concatenate([sp[32:], sp[:32]])
    sin_eff = (sin_e * swap_sp[None, :]).astype(np.float32)
    # device layout [128, SS*DH]: [p, b*64+c] = tbl[b*128+p, c]
    cosd = np.ascontiguousarray(
        cos_eff.reshape(SS, 128, DH).transpose(1, 0, 2).reshape(128, SS * DH))
    sind = np.ascontiguousarray(
        sin_eff.reshape(SS, 128, DH).transpose(1, 0, 2).reshape(128, SS * DH))

    # causal triangle for the diagonal 128-blocks: keep sjl <= sil
    sjl = np.arange(128)[:, None]
    sil = np.arange(128)[None, :]
    trid = (sjl <= sil).astype(np.float32)

    # indicator for denominator broadcast: ind8[k, h*64+m] = (k == h)
    ind8 = np.zeros((8, 512), dtype=np.float32)
    for h in range(8):
        ind8[h, h * 64:(h + 1) * 64] = 1.0

    in_maps = []
    for c in range(NCORES):
        b, t = divmod(c, TP)
        e0 = t * E
        in_maps.append({
            "xT": np.ascontiguousarray(x[b].T).astype(BF16),
            "wqT": np.ascontiguousarray(wq_n[e0:e0 + E].T).astype(BF16),
            "wkT": np.ascontiguousarray(wk_n[e0:e0 + E].T).astype(BF16),
            "wvT": np.ascontiguousarray(wv_n[e0:e0 + E].T).astype(BF16),
            "woT": np.ascontiguousarray(wo_n[:, e0:e0 + E].T).astype(BF16),
            "cosd": cosd.astype(BF16), "sind": sind.astype(BF16),
            "trid": trid.astype(BF16), "ind8d": ind8.astype(BF16),
        })
    return in_maps


def _install_profile_hook():
    """antenv.axon_hooks is absent in this image; shim it and register the
    ctypes NTFF hook against /opt/axon/libaxon_pjrt.so (mirrors trn_boot)."""
    import types
    import ctypes
    import contextlib

    try:
        from antenv.axon_hooks import get_axon_ntff_profile_hook  # noqa
        return
    except ImportError:
        pass
    import antenv
    mod = types.ModuleType("antenv.axon_hooks")
    state = {}
    mod.set_axon_ntff_profile_hook = lambda h: state.__setitem__("h", h)
    mod.get_axon_ntff_profile_hook = lambda: state.get("h")
    sys.modules["antenv.axon_hooks"] = mod
    antenv.axon_hooks = mod

    so_path = "/opt/axon/libaxon_pjrt.so"
    lib = ctypes.CDLL(so_path)
    if not hasattr(lib, "axon_start_nrt_profile"):
        return
    lib.axon_start_nrt_profile.argtypes = [
        ctypes.POINTER(ctypes.c_int64), ctypes.c_size_t]
    lib.axon_start_nrt_profile.restype = ctypes.c_int64
    lib.axon_stop_nrt_profile.argtypes = [ctypes.c_char_p]
    lib.axon_stop_nrt_profile.restype = ctypes.c_int64

    @contextlib.contextmanager
    def _hook(output_dir, device_ids):
        import jax
        jax.devices()
        if device_ids:
            ids = (ctypes.c_int64 * len(device_ids))(*device_ids)
            rc = lib.axon_start_nrt_profile(ids, len(device_ids))
        else:
            rc = lib.axon_start_nrt_profile(None, 0)
        if rc != 0:
            raise RuntimeError(f"axon_start_nrt_profile rc={rc}")
        try:
            yield
        finally:
            n = lib.axon_stop_nrt_profile(str(output_dir).encode())
            print(f"profile: {n} file(s) written to {output_dir}",
                  file=sys.stderr)

    mod.set_axon_ntff_profile_hook(_hook)


def kernel(x, wq, wk, wv, wo, qk_scale, _profile=False):
    from concourse.bass_utils import run_bass_kernel_spmd

    if _profile:
        _install_profile_hook()

    if "nc" not in _CACHE:
        nc = _build_program()
        nc.finalize()
        _CACHE["nc"] = nc
    nc = _CACHE["nc"]
    in_maps = _host_prep(np.asarray(x), np.asarray(wq), np.asarray(wk),
                         np.asarray(wv), np.asarray(wo), np.asarray(qk_scale))
    res = run_bass_kernel_spmd(nc, in_maps, core_ids=list(range(NCORES)),
                               trace=_profile)
    outs = res.results
    y = np.empty((B, S, DIM), dtype=np.float32)
    for b in range(B):
        y[b] = sum(outs[b * TP + t]["Y"] for t in range(TP))
    if _profile:
        _CACHE["last_exec_time_ns"] = res.exec_time_ns
        _CACHE["last_profile"] = res.profile_json
    return y


# revision 21
# speedup vs baseline: 1.1968x; 1.0867x over previous
"""nn_Attention Trainium2 Bass kernel (v2 — interleaved pipeline).

Full attention forward: x->(q,k,v) with l2-normalized weights, per-head-dim
l2 norm + learned qk scale, interleaved RoPE, causal SDPA, output projection
with column-l2-normalized wo.

Sharding: TP=4 over heads (8 heads/core) x DP=2 over batch across 8 cores.
Each core computes a partial [2048, 2048] output for its batch; host sums
the 4 TP partials per batch.

v2 changes vs v1:
- single interleaved loop per 512-row block: proj -> attention -> yproj,
  so DVE rope work, Act exp work and PE matmuls overlap across phases.
- q/k transposes via DMA xbar (dma_start_transpose) instead of PE
  transposes + DVE copies.
- causal mask as a single 128x128 triangle multiply on the Pool engine.
- lg/pv matmuls trimmed to the live columns on diagonal blocks.
- softmax denominators: v's 65th ones-column -> psum row 64 -> stashed ->
  gathered by DMA -> PE-transposed to si-partition layout -> one cheap
  [128,32] reciprocal -> transposed back -> rank-8 indicator matmul
  broadcast (replaces 3.3us-per-call wide DVE reciprocals).
- x streamed per 512-column block (2-deep) instead of fully resident.
- yproj results DMA'd directly from PSUM to DRAM.
"""
import sys
import os
import math
from contextlib import ExitStack

sys.path.insert(0, "/opt/trn_rl_repo")

import numpy as np
import ml_dtypes

BF16 = ml_dtypes.bfloat16

B, S, DIM = 2, 2048, 2048
HEADS, DH = 32, 64
THETA = 10000.0
NCORES = 8
TP = 4             # head-parallel ways
HPC = HEADS // TP  # heads per core = 8
E = HPC * DH       # per-core qkv width = 512
ET = E // 128      # e-tiles per core = 4
DT = DIM // 128    # contraction d-tiles = 16
SB = S // 512      # 512-wide seq blocks = 4
SS = S // 128      # 128-wide seq blocks = 16

_CACHE = {}


def _l2n(w, axis):
    n = np.sqrt((w.astype(np.float64) ** 2).sum(axis=axis, keepdims=True))
    n = np.maximum(n, 1e-12)
    return (w / n).astype(np.float32)


def _build_program():
    import concourse.bass as bass
    from concourse import bacc
    import concourse.mybir as mybir
    import concourse.tile as tile
    from concourse.masks import make_identity

    f32 = mybir.dt.float32
    bf16 = mybir.dt.bfloat16
    AF = mybir.ActivationFunctionType
    AX = mybir.AxisListType
    OP = mybir.AluOpType

    nc = bacc.Bacc("TRN2", target_bir_lowering=False)

    xT = nc.dram_tensor("xT", [DIM, S], bf16, kind="ExternalInput")
    wqT = nc.dram_tensor("wqT", [DIM, E], bf16, kind="ExternalInput")
    wkT = nc.dram_tensor("wkT", [DIM, E], bf16, kind="ExternalInput")
    wvT = nc.dram_tensor("wvT", [DIM, E], bf16, kind="ExternalInput")
    woT = nc.dram_tensor("woT", [E, DIM], bf16, kind="ExternalInput")
    cosd = nc.dram_tensor("cosd", [128, SS * DH], bf16, kind="ExternalInput")
    sind = nc.dram_tensor("sind", [128, SS * DH], bf16, kind="ExternalInput")
    trid = nc.dram_tensor("trid", [128, 128], bf16, kind="ExternalInput")
    ind8d = nc.dram_tensor("ind8d", [8, 512], bf16, kind="ExternalInput")
    Y = nc.dram_tensor("Y", [S, DIM], f32, kind="ExternalOutput")

    with tile.TileContext(nc) as tc, ExitStack() as ctx:
        const = ctx.enter_context(tc.tile_pool(name="const", bufs=1))
        wpool = ctx.enter_context(tc.tile_pool(name="wpool", bufs=4))
        xpool = ctx.enter_context(tc.tile_pool(name="xpool", bufs=2))
        qkv = ctx.enter_context(tc.tile_pool(name="qkv", bufs=1))
        work = ctx.enter_context(tc.tile_pool(name="work", bufs=1))
        expool = ctx.enter_context(tc.tile_pool(name="expool", bufs=4))
        psA = ctx.enter_context(
            tc.tile_pool(name="psA", bufs=4, space="PSUM"))
        psL = ctx.enter_context(
            tc.tile_pool(name="psL", bufs=2, space="PSUM"))

        # --- weights (wq first, quartered, so proj can start early) ---
        wq_sb = [wpool.tile([128, 4, E], bf16, tag=f"wq{j}", bufs=1, name=f"wq{j}")
                 for j in range(4)]
        wk_sb = wpool.tile([128, DT, E], bf16, tag="wk", bufs=1)
        wv_sb = wpool.tile([128, DT, E], bf16, tag="wv", bufs=1)
        wo_sb = wpool.tile([128, ET, DIM], bf16, tag="wo", bufs=1)
        wqr = wqT.rearrange("(t p) e -> p t e", p=128)

        xtiles = {}

        def load_x(st):
            ts = [xpool.tile([128, 4, 512], bf16, tag=f"x{j}", bufs=2,
                             name=f"xst{st}_{j}") for j in range(4)]
            src = xT[:, st * 512:(st + 1) * 512].rearrange(
                "(t p) s -> p t s", p=128)
            for j in range(4):
                nc.sync.dma_start(ts[j], src[:, j * 4:(j + 1) * 4, :])
            return ts

        # interleave wq quarters with x quarters so dt=0..3 can start early
        x0src = xT[:, 0:512].rearrange("(t p) s -> p t s", p=128)
        x0 = [xpool.tile([128, 4, 512], bf16, tag=f"x{j}", bufs=2,
                         name=f"xst0_{j}") for j in range(4)]
        for j in range(4):
            nc.sync.dma_start(wq_sb[j], wqr[:, j * 4:(j + 1) * 4, :])
            nc.sync.dma_start(x0[j], x0src[:, j * 4:(j + 1) * 4, :])
        xtiles[0] = x0
        nc.sync.dma_start(wk_sb, wkT.rearrange("(t p) e -> p t e", p=128))
        nc.sync.dma_start(wv_sb, wvT.rearrange("(t p) e -> p t e", p=128))

        # --- constants ---
        cos_sb = const.tile([128, SS, DH], bf16)
        sin_sb = const.tile([128, SS, DH], bf16)
        nc.sync.dma_start(cos_sb, cosd.rearrange("p (b d) -> p b d", d=DH))
        nc.sync.dma_start(sin_sb, sind.rearrange("p (b d) -> p b d", d=DH))
        tri = const.tile([128, 128], bf16)
        nc.sync.dma_start(tri, trid[:, :])
        ind8 = const.tile([8, 512], bf16)
        nc.sync.dma_start(ind8, ind8d[:, :])
        nc.sync.dma_start(wo_sb, woT.rearrange("(t p) e -> p t e", p=128))
        identf = const.tile([128, 128], f32)
        make_identity(nc, identf)
        ident = const.tile([128, 128], bf16)
        make_identity(nc, ident)

        # --- persistent activations ---
        qTall = qkv.tile([128, ET, S], bf16, tag="qT")
        kTall = qkv.tile([128, ET, S], bf16, tag="kT")
        v_sb = qkv.tile([128, SS, HPC, 66], bf16, tag="v")
        stash = qkv.tile([65, HPC, 512], bf16, tag="stash")
        nc.vector.memset(v_sb[:, :, :, 64:66], 1.0)

        def norm_rope(ps, dstT, st, su):
            """psum [si,e] natural -> per-head l2norm, rope, bf16,
            -> DMA-transpose into dstT columns."""
            sblk = st * 4 + su
            sq = work.tile([128, E], bf16, tag="sq", bufs=2)
            nc.scalar.square(sq, ps)
            ssq = work.tile([128, HPC], f32, tag="ssq", bufs=2)
            nc.vector.tensor_reduce(
                ssq, sq.rearrange("p (h d) -> p h d", d=DH),
                axis=AX.X, op=OP.add)
            # rsqrt via magic-number seed + 2 Newton iterations (DVE only;
            # keeps the Act engine free of sqrt so its activation table
            # never leaves the exp set)
            inv = work.tile([128, HPC], f32, tag="inv", bufs=2)
            ssq_i = ssq.bitcast(mybir.dt.int32)
            inv_i = inv.bitcast(mybir.dt.int32)
            nc.vector.tensor_scalar(inv_i, ssq_i, 1, None,
                                    op0=OP.arith_shift_right)
            nc.vector.tensor_scalar(inv_i, inv_i, 0x5f3759df, -1,
                                    op0=OP.subtract, op1=OP.mult)
            y2 = work.tile([128, HPC], f32, tag="y2", bufs=2)
            for _ in range(2):
                nc.vector.tensor_mul(y2, inv, inv)
                nc.vector.scalar_tensor_tensor(
                    y2, ssq, -0.5, y2, op0=OP.mult, op1=OP.mult)
                nc.vector.tensor_scalar(y2, y2, 1.5, None, op0=OP.add)
                nc.vector.tensor_mul(inv, inv, y2)
            qn = work.tile([128, HPC, DH], bf16, tag="qn", bufs=2)
            nc.vector.tensor_mul(
                qn, ps.rearrange("p (h d) -> p h d", d=DH),
                inv.unsqueeze(2).broadcast_to([128, HPC, DH]))
            cosb = cos_sb[:, sblk:sblk + 1, :].broadcast_to([128, HPC, DH])
            sinb = sin_sb[:, sblk:sblk + 1, :].broadcast_to([128, HPC, DH])
            rot = work.tile([128, HPC, 2, 32], bf16, tag="rot", bufs=2)
            qn4 = qn.rearrange("p h (t u) -> p h t u", u=32)
            nc.vector.tensor_copy(rot[:, :, 0:1, :], qn4[:, :, 1:2, :])
            nc.vector.tensor_copy(rot[:, :, 1:2, :], qn4[:, :, 0:1, :])
            nc.vector.tensor_mul(rot.rearrange("p h t u -> p h (t u)"),
                                 rot.rearrange("p h t u -> p h (t u)"), sinb)
            nc.vector.tensor_mul(qn, qn, cosb)
            qo = work.tile([128, E], bf16, tag="qo", bufs=2)
            nc.vector.tensor_add(
                qo, qn.rearrange("p h d -> p (h d)"),
                rot.rearrange("p h t u -> p (h t u)"))
            nc.sync.dma_start_transpose(
                dstT[:, :, sblk * 128:(sblk + 1) * 128], qo)

        def proj_half(w_sb, kind, st, s0):
            xt = xtiles[st]
            quartered = isinstance(w_sb, list)
            prs = [psA.tile([128, E], f32, tag="ps",
                            name=f"p{kind}{st}_{s0 + j}")
                   for j in range(2)]
            for dt in range(DT):
                if quartered:
                    wslice = w_sb[dt // 4][:, dt % 4, :]
                else:
                    wslice = w_sb[:, dt, :]
                for j in range(2):
                    su = s0 + j
                    nc.tensor.matmul(
                        prs[j],
                        xt[dt // 4][:, dt % 4, su * 128:(su + 1) * 128],
                        wslice,
                        start=(dt == 0), stop=(dt == DT - 1))
            for j in range(2):
                su = s0 + j
                if kind == "v":
                    nc.vector.tensor_copy(
                        v_sb[:, st * 4 + su, :, 0:64],
                        prs[j].rearrange("p (h d) -> p h d", d=DH))
                else:
                    norm_rope(prs[j], qTall if kind == "q" else kTall,
                              st, su)

        def proj_all(st):
            for w_sb, kind in ((wq_sb, "q"), (wk_sb, "k"), (wv_sb, "v")):
                for s0 in (0, 2):
                    proj_half(w_sb, kind, st, s0)

        def attn_pair(i, ha, hb):
            """Head-paired attention: heads (h, h+2) share PE tile config
            (same hp), so lg and pv matmuls run in same-config groups of 4
            with alternating PSUM banks."""
            last = 4 * i + 3
            npr = 2 * (i + 1)
            if True:
                hp = (ha % 2) * 64
                ets = {ha: ha // 2, hb: hb // 2}
                pvs = {h: psA.tile([128, 512], f32, tag="ps",
                                   name=f"pv{i}_{h}")
                       for h in (ha, hb)}
                lgs = {}

                def lg4(p):
                    for h in (ha, hb):
                        lgs[(h, p)] = psL.tile(
                            [128, 2, 512], f32, tag="lg",
                            name=f"lg{i}_{h}_{p}")
                    for b in range(2):
                        sjb = 2 * p + b
                        r = sjb - 4 * i
                        c0 = r * 128 if r > 0 else 0
                        for h in (ha, hb):
                            nc.tensor.matmul(
                                lgs[(h, p)][:, b, c0:],
                                kTall[hp:hp + 64, ets[h],
                                      sjb * 128:(sjb + 1) * 128],
                                qTall[hp:hp + 64, ets[h],
                                      i * 512 + c0:(i + 1) * 512],
                                start=True, stop=True)

                lg4(0)
                for p in range(npr):
                    exs = {}
                    for h in (ha, hb):
                        lg2 = lgs.pop((h, p))
                        ex = expool.tile([128, 2, 512], bf16, tag="ex",
                                         name=f"ex{i}_{h}_{p}")
                        if 2 * p - 4 * i >= 0:  # diagonal pair: match trim
                            for b in range(2):
                                c0 = max(0, (2 * p + b - 4 * i)) * 128
                                nc.scalar.activation(ex[:, b, c0:],
                                                     lg2[:, b, c0:], AF.Exp)
                        else:
                            nc.scalar.activation(ex, lg2, AF.Exp)
                        exs[h] = ex
                    if p + 1 < npr:
                        lg4(p + 1)
                    for b in range(2):
                        sjb = 2 * p + b
                        r = sjb - 4 * i
                        c0 = r * 128 if r > 0 else 0
                        if r >= 0:
                            for h in (ha, hb):
                                nc.gpsimd.tensor_mul(
                                    exs[h][:, b, r * 128:(r + 1) * 128],
                                    exs[h][:, b, r * 128:(r + 1) * 128],
                                    tri)
                        for h in (ha, hb):
                            nc.tensor.matmul(
                                pvs[h][0:66, c0:],
                                v_sb[:, sjb, h, :],
                                exs[h][:, b, c0:],
                                start=(sjb == 0), stop=(sjb == last))
                for h in (ha, hb):
                    nc.vector.tensor_copy(stash[:, h, :], pvs[h][0:65, :])

        def normalize_gather(i):
            den = work.tile([8, 512], bf16, tag="den", bufs=2)
            nc.scalar.dma_start(den, stash[64:65, :, :])
            return den

        def normalize_recip(i, den):
            """den rows -> si-partition layout via PE transposes -> one
            cheap [128,32] DVE reciprocal."""
            invT = psA.tile([128, 32], bf16, tag="ps")
            for c in range(4):
                nc.tensor.transpose(
                    invT[:, c * 8:(c + 1) * 8],
                    den[:, c * 128:(c + 1) * 128], ident[0:8, 0:8])
            inv_sb = work.tile([128, 32], f32, tag="invsb", bufs=2)
            nc.vector.reciprocal(inv_sb, invT)
            return inv_sb

        def normalize_apply(i, inv_sb):
            """transpose back to row layout, rank-8 indicator broadcast,
            per-head mul into qTall."""
            invrow = psA.tile([8, 4, 128], f32, tag="ps")
            for c in range(4):
                nc.tensor.transpose(
                    invrow[:, c, :], inv_sb[:, c * 8:(c + 1) * 8], identf)
            inv_row = work.tile([8, 512], bf16, tag="invrowsb", bufs=2)
            nc.vector.tensor_copy(
                inv_row, invrow.rearrange("p c j -> p (c j)"))
            for h in range(HPC):
                et, hp = h // 2, (h % 2) * 64
                bc = psA.tile([64, 512], f32, tag="ps", name=f"bc{i}_{h}")
                nc.tensor.matmul(bc, ind8[:, h * 64:(h + 1) * 64], inv_row,
                                 start=True, stop=True)
                nc.vector.tensor_mul(
                    qTall[hp:hp + 64, et, i * 512:(i + 1) * 512],
                    stash[0:64, h, :], bc)

        def yproj_block(i):
            for ib in range(4 * i, 4 * i + 4):
                for nd0 in (0, 2):
                    pss = [psA.tile([128, 512], f32, tag="ps",
                                    name=f"y{ib}_{nd0 + j}")
                           for j in range(2)]
                    for ket in range(ET):
                        for j in range(2):
                            nd = nd0 + j
                            nc.tensor.matmul(
                                pss[j],
                                qTall[:, ket, ib * 128:(ib + 1) * 128],
                                wo_sb[:, ket, nd * 512:(nd + 1) * 512],
                                start=(ket == 0), stop=(ket == ET - 1))
                    for j in range(2):
                        nd = nd0 + j
                        ys = work.tile([128, 512], f32, tag="ys", bufs=2)
                        if nd % 2 == 0:
                            nc.vector.tensor_copy(ys, pss[j])
                        else:
                            nc.scalar.copy(ys, pss[j])
                        nc.sync.dma_start(
                            Y[ib * 128:(ib + 1) * 128,
                              nd * 512:(nd + 1) * 512], ys)

        proj_all(0)
        pairs = ((0, 2), (4, 6), (1, 3), (5, 7))
        halves = ((wq_sb, "q", 0), (wq_sb, "q", 2), (wk_sb, "k", 0),
                  (wk_sb, "k", 2), (wv_sb, "v", 0), (wv_sb, "v", 2))
        for st in range(SB):
            nxt = st + 1 < SB
            if nxt:
                xtiles[st + 1] = load_x(st + 1)
            for idx, (ha, hb) in enumerate(pairs):
                attn_pair(st, ha, hb)
                if nxt:
                    w, kind, s0 = halves[idx]
                    proj_half(w, kind, st + 1, s0)
            den = normalize_gather(st)
            inv_sb = normalize_recip(st, den)
            if nxt:
                proj_half(*halves[4][:2], st + 1, halves[4][2])
            normalize_apply(st, inv_sb)
            if nxt:
                proj_half(*halves[5][:2], st + 1, halves[5][2])
            yproj_block(st)

    return nc


def _host_prep(x, wq, wk, wv, wo, qk_scale):
    """Returns per-core input dicts."""
    perm = np.concatenate([np.arange(0, DH, 2), np.arange(1, DH, 2)])
    wq_n = _l2n(wq, -1).reshape(HEADS, DH, DIM)[:, perm, :].reshape(HEADS * DH, DIM)
    wk_n = _l2n(wk, -1).reshape(HEADS, DH, DIM)[:, perm, :].reshape(HEADS * DH, DIM)
    wv_n = _l2n(wv, -1)
    wo_n = _l2n(wo, 0)
    sp = qk_scale.astype(np.float64)[perm]

    # rope tables with qk_scale folded in; permuted-block layout
    half = np.arange(0, DH, 2)
    freqs = 1.0 / (THETA ** (half.astype(np.float64) / DH))      # (32,)
    ang = np.arange(S, dtype=np.float64)[:, None] * freqs[None]  # (S, 32)
    cos_h, sin_h = np.cos(ang), np.sin(ang)
    cos_p = np.concatenate([cos_h, cos_h], 1)                    # (S, 64)
    sin_e = np.concatenate([-sin_h, sin_h], 1)
    cos_eff = (cos_p * sp[None, :]).astype(np.float32)
    swap_sp = np.concatenate([sp[32:], sp[:32]])
    sin_eff = (sin_e * swap_sp[None, :]).astype(np.float32)
    # device layout [128, SS*DH]: [p, b*64+c] = tbl[b*128+p, c]
    cosd = np.ascontiguousarray(
        cos_eff.reshape(SS, 128, DH).transpose(1, 0, 2).reshape(128, SS * DH))
    sind = np.ascontiguousarray(
        sin_eff.reshape(SS, 128, DH).transpose(1, 0, 2).reshape(128, SS * DH))

    # causal triangle for the diagonal 128-blocks: keep sjl <= sil
    sjl = np.arange(128)[:, None]
    sil = np.arange(128)[None, :]
    trid = (sjl <= sil).astype(np.float32)

    # indicator for denominator broadcast: ind8[k, h*64+m] = (k == h)
    ind8 = np.zeros((8, 512), dtype=np.float32)
    for h in range(8):
        ind8[h, h * 64:(h + 1) * 64] = 1.0

    in_maps = []
    for c in range(NCORES):
        b, t = divmod(c, TP)
        e0 = t * E
        in_maps.append({
            "xT": np.ascontiguousarray(x[b].T).astype(BF16),
            "wqT": np.ascontiguousarray(wq_n[e0:e0 + E].T).astype(BF16),
            "wkT": np.ascontiguousarray(wk_n[e0:e0 + E].T).astype(BF16),
            "wvT": np.ascontiguousarray(wv_n[e0:e0 + E].T).astype(BF16),
            "woT": np.ascontiguousarray(wo_n[:, e0:e0 + E].T).astype(BF16),
            "cosd": cosd.astype(BF16), "sind": sind.astype(BF16),
            "trid": trid.astype(BF16), "ind8d": ind8.astype(BF16),
        })
    return in_maps


def _install_profile_hook():
    """antenv.axon_hooks is absent in this image; shim it and register the
    ctypes NTFF hook against /opt/axon/libaxon_pjrt.so (mirrors trn_boot)."""
    import types
    import ctypes
    import contextlib

    try:
        from antenv.axon_hooks import get_axon_ntff_profile_hook  # noqa
        return
    except ImportError:
        pass
    import antenv
    mod = types.ModuleType("antenv.axon_hooks")
    state = {}
    mod.set_axon_ntff_profile_hook = lambda h: state.__setitem__("h", h)
    mod.get_axon_ntff_profile_hook = lambda: state.get("h")
    sys.modules["antenv.axon_hooks"] = mod
    antenv.axon_hooks = mod

    so_path = "/opt/axon/libaxon_pjrt.so"
    lib = ctypes.CDLL(so_path)
    if not hasattr(lib, "axon_start_nrt_profile"):
        return
    lib.axon_start_nrt_profile.argtypes = [
        ctypes.POINTER(ctypes.c_int64), ctypes.c_size_t]
    lib.axon_start_nrt_profile.restype = ctypes.c_int64
    lib.axon_stop_nrt_profile.argtypes = [ctypes.c_char_p]
    lib.axon_stop_nrt_profile.restype = ctypes.c_int64

    @contextlib.contextmanager
    def _hook(output_dir, device_ids):
        import jax
        jax.devices()
        if device_ids:
            ids = (ctypes.c_int64 * len(device_ids))(*device_ids)
            rc = lib.axon_start_nrt_profile(ids, len(device_ids))
        else:
            rc = lib.axon_start_nrt_profile(None, 0)
        if rc != 0:
            raise RuntimeError(f"axon_start_nrt_profile rc={rc}")
        try:
            yield
        finally:
            n = lib.axon_stop_nrt_profile(str(output_dir).encode())
            print(f"profile: {n} file(s) written to {output_dir}",
                  file=sys.stderr)

    mod.set_axon_ntff_profile_hook(_hook)


def kernel(x, wq, wk, wv, wo, qk_scale, _profile=False):
    from concourse.bass_utils import run_bass_kernel_spmd

    if _profile:
        _install_profile_hook()

    if "nc" not in _CACHE:
        nc = _build_program()
        nc.finalize()
        _CACHE["nc"] = nc
    nc = _CACHE["nc"]
    in_maps = _host_prep(np.asarray(x), np.asarray(wq), np.asarray(wk),
                         np.asarray(wv), np.asarray(wo), np.asarray(qk_scale))
    res = run_bass_kernel_spmd(nc, in_maps, core_ids=list(range(NCORES)),
                               trace=_profile)
    outs = res.results
    y = np.empty((B, S, DIM), dtype=np.float32)
    for b in range(B):
        y[b] = sum(outs[b * TP + t]["Y"] for t in range(TP))
    if _profile:
        _CACHE["last_exec_time_ns"] = res.exec_time_ns
        _CACHE["last_profile"] = res.profile_json
    return y


# revision 22
# speedup vs baseline: 1.3542x; 1.1315x over previous
"""nn_Attention Trainium2 Bass kernel (v2 — interleaved pipeline).

Full attention forward: x->(q,k,v) with l2-normalized weights, per-head-dim
l2 norm + learned qk scale, interleaved RoPE, causal SDPA, output projection
with column-l2-normalized wo.

Sharding: TP=4 over heads (8 heads/core) x DP=2 over batch across 8 cores.
Each core computes a partial [2048, 2048] output for its batch; host sums
the 4 TP partials per batch.

v2 changes vs v1:
- single interleaved loop per 512-row block: proj -> attention -> yproj,
  so DVE rope work, Act exp work and PE matmuls overlap across phases.
- q/k transposes via DMA xbar (dma_start_transpose) instead of PE
  transposes + DVE copies.
- causal mask as a single 128x128 triangle multiply on the Pool engine.
- lg/pv matmuls trimmed to the live columns on diagonal blocks.
- softmax denominators: v's 65th ones-column -> psum row 64 -> stashed ->
  gathered by DMA -> PE-transposed to si-partition layout -> one cheap
  [128,32] reciprocal -> transposed back -> rank-8 indicator matmul
  broadcast (replaces 3.3us-per-call wide DVE reciprocals).
- x streamed per 512-column block (2-deep) instead of fully resident.
- yproj results DMA'd directly from PSUM to DRAM.
"""
import sys
import os
import math
from contextlib import ExitStack

sys.path.insert(0, "/opt/trn_rl_repo")

import numpy as np
import ml_dtypes

BF16 = ml_dtypes.bfloat16

B, S, DIM = 2, 2048, 2048
HEADS, DH = 32, 64
THETA = 10000.0
NCORES = 8
TP = 4             # head-parallel ways
HPC = HEADS // TP  # heads per core = 8
E = HPC * DH       # per-core qkv width = 512
ET = E // 128      # e-tiles per core = 4
DT = DIM // 128    # contraction d-tiles = 16
SB = S // 512      # 512-wide seq blocks = 4
SS = S // 128      # 128-wide seq blocks = 16

_CACHE = {}


def _l2n(w, axis):
    n = np.sqrt((w.astype(np.float64) ** 2).sum(axis=axis, keepdims=True))
    n = np.maximum(n, 1e-12)
    return (w / n).astype(np.float32)


def _build_program():
    import concourse.bass as bass
    from concourse import bacc
    import concourse.mybir as mybir
    import concourse.tile as tile
    from concourse.masks import make_identity

    f32 = mybir.dt.float32
    bf16 = mybir.dt.bfloat16
    AF = mybir.ActivationFunctionType
    AX = mybir.AxisListType
    OP = mybir.AluOpType

    nc = bacc.Bacc("TRN2", target_bir_lowering=False)

    xT = nc.dram_tensor("xT", [DIM, S], bf16, kind="ExternalInput")
    wqT = nc.dram_tensor("wqT", [DIM, E], bf16, kind="ExternalInput")
    wkT = nc.dram_tensor("wkT", [DIM, E], bf16, kind="ExternalInput")
    wvT = nc.dram_tensor("wvT", [DIM, E], bf16, kind="ExternalInput")
    woT = nc.dram_tensor("woT", [E, DIM], bf16, kind="ExternalInput")
    cosd = nc.dram_tensor("cosd", [128, SS * DH], bf16, kind="ExternalInput")
    sind = nc.dram_tensor("sind", [128, SS * DH], bf16, kind="ExternalInput")
    trid = nc.dram_tensor("trid", [128, 128], bf16, kind="ExternalInput")
    ind8d = nc.dram_tensor("ind8d", [8, 512], bf16, kind="ExternalInput")
    Y = nc.dram_tensor("Y", [S, DIM], f32, kind="ExternalOutput")

    with tile.TileContext(nc) as tc, ExitStack() as ctx:
        const = ctx.enter_context(tc.tile_pool(name="const", bufs=1))
        wpool = ctx.enter_context(tc.tile_pool(name="wpool", bufs=4))
        xpool = ctx.enter_context(tc.tile_pool(name="xpool", bufs=2))
        qkv = ctx.enter_context(tc.tile_pool(name="qkv", bufs=1))
        work = ctx.enter_context(tc.tile_pool(name="work", bufs=1))
        expool = ctx.enter_context(tc.tile_pool(name="expool", bufs=4))
        psA = ctx.enter_context(
            tc.tile_pool(name="psA", bufs=4, space="PSUM"))
        psL = ctx.enter_context(
            tc.tile_pool(name="psL", bufs=2, space="PSUM"))

        # --- weights (wq first, quartered, so proj can start early) ---
        wq_sb = [wpool.tile([128, 4, E], bf16, tag=f"wq{j}", bufs=1, name=f"wq{j}")
                 for j in range(4)]
        wk_sb = wpool.tile([128, DT, E], bf16, tag="wk", bufs=1)
        wv_sb = wpool.tile([128, DT, E], bf16, tag="wv", bufs=1)
        wo_sb = wpool.tile([128, ET, DIM], bf16, tag="wo", bufs=1)
        wqr = wqT.rearrange("(t p) e -> p t e", p=128)

        xtiles = {}

        def load_x(st):
            ts = [xpool.tile([128, 4, 512], bf16, tag=f"x{j}", bufs=2,
                             name=f"xst{st}_{j}") for j in range(4)]
            src = xT[:, st * 512:(st + 1) * 512].rearrange(
                "(t p) s -> p t s", p=128)
            for j in range(4):
                nc.sync.dma_start(ts[j], src[:, j * 4:(j + 1) * 4, :])
            return ts

        # interleave wq quarters with x quarters so dt=0..3 can start early
        x0src = xT[:, 0:512].rearrange("(t p) s -> p t s", p=128)
        x0 = [xpool.tile([128, 4, 512], bf16, tag=f"x{j}", bufs=2,
                         name=f"xst0_{j}") for j in range(4)]
        for j in range(4):
            nc.sync.dma_start(wq_sb[j], wqr[:, j * 4:(j + 1) * 4, :])
            nc.sync.dma_start(x0[j], x0src[:, j * 4:(j + 1) * 4, :])
        xtiles[0] = x0
        nc.sync.dma_start(wk_sb, wkT.rearrange("(t p) e -> p t e", p=128))
        nc.sync.dma_start(wv_sb, wvT.rearrange("(t p) e -> p t e", p=128))

        # --- constants ---
        cos_sb = const.tile([128, SS, DH], bf16)
        sin_sb = const.tile([128, SS, DH], bf16)
        nc.sync.dma_start(cos_sb, cosd.rearrange("p (b d) -> p b d", d=DH))
        nc.sync.dma_start(sin_sb, sind.rearrange("p (b d) -> p b d", d=DH))
        tri = const.tile([128, 128], bf16)
        nc.sync.dma_start(tri, trid[:, :])
        ind8 = const.tile([8, 512], bf16)
        nc.sync.dma_start(ind8, ind8d[:, :])
        nc.sync.dma_start(wo_sb, woT.rearrange("(t p) e -> p t e", p=128))
        identf = const.tile([128, 128], f32)
        make_identity(nc, identf)
        ident = const.tile([128, 128], bf16)
        make_identity(nc, ident)

        # --- persistent activations ---
        qTall = qkv.tile([128, ET, S], bf16, tag="qT")
        kTall = qkv.tile([128, ET, S], bf16, tag="kT")
        v_sb = qkv.tile([128, SS, HPC, 66], bf16, tag="v")
        stash = qkv.tile([65, HPC, 512], bf16, tag="stash")
        nc.vector.memset(v_sb[:, :, :, 64:66], 1.0)

        def norm_rope(ps, dstT, st, su):
            """psum [si,e] natural -> per-head l2norm, rope, bf16,
            -> DMA-transpose into dstT columns."""
            sblk = st * 4 + su
            sq = work.tile([128, E], bf16, tag="sq", bufs=2)
            nc.scalar.square(sq, ps)
            ssq = work.tile([128, HPC], f32, tag="ssq", bufs=2)
            nc.vector.tensor_reduce(
                ssq, sq.rearrange("p (h d) -> p h d", d=DH),
                axis=AX.X, op=OP.add)
            # rsqrt via magic-number seed + 2 Newton iterations (DVE only;
            # keeps the Act engine free of sqrt so its activation table
            # never leaves the exp set)
            inv = work.tile([128, HPC], f32, tag="inv", bufs=2)
            ssq_i = ssq.bitcast(mybir.dt.int32)
            inv_i = inv.bitcast(mybir.dt.int32)
            nc.vector.tensor_scalar(inv_i, ssq_i, 1, None,
                                    op0=OP.arith_shift_right)
            nc.vector.tensor_scalar(inv_i, inv_i, 0x5f3759df, -1,
                                    op0=OP.subtract, op1=OP.mult)
            y2 = work.tile([128, HPC], f32, tag="y2", bufs=2)
            for _ in range(2):
                nc.vector.tensor_mul(y2, inv, inv)
                nc.vector.scalar_tensor_tensor(
                    y2, ssq, -0.5, y2, op0=OP.mult, op1=OP.mult)
                nc.vector.tensor_scalar(y2, y2, 1.5, None, op0=OP.add)
                nc.vector.tensor_mul(inv, inv, y2)
            qn = work.tile([128, HPC, DH], bf16, tag="qn", bufs=2)
            nc.vector.tensor_mul(
                qn, ps.rearrange("p (h d) -> p h d", d=DH),
                inv.unsqueeze(2).broadcast_to([128, HPC, DH]))
            cosb = cos_sb[:, sblk:sblk + 1, :].broadcast_to([128, HPC, DH])
            sinb = sin_sb[:, sblk:sblk + 1, :].broadcast_to([128, HPC, DH])
            rot = work.tile([128, HPC, 2, 32], bf16, tag="rot", bufs=2)
            qn4 = qn.rearrange("p h (t u) -> p h t u", u=32)
            nc.vector.tensor_copy(rot[:, :, 0:1, :], qn4[:, :, 1:2, :])
            nc.vector.tensor_copy(rot[:, :, 1:2, :], qn4[:, :, 0:1, :])
            nc.vector.tensor_mul(rot.rearrange("p h t u -> p h (t u)"),
                                 rot.rearrange("p h t u -> p h (t u)"), sinb)
            nc.vector.tensor_mul(qn, qn, cosb)
            qo = work.tile([128, E], bf16, tag="qo", bufs=2)
            nc.vector.tensor_add(
                qo, qn.rearrange("p h d -> p (h d)"),
                rot.rearrange("p h t u -> p (h t u)"))
            nc.sync.dma_start_transpose(
                dstT[:, :, sblk * 128:(sblk + 1) * 128], qo)

        def proj_half(w_sb, kind, st, s0):
            xt = xtiles[st]
            quartered = isinstance(w_sb, list)
            prs = [psA.tile([128, E], f32, tag="ps",
                            name=f"p{kind}{st}_{s0 + j}")
                   for j in range(2)]
            for dt in range(DT):
                if quartered:
                    wslice = w_sb[dt // 4][:, dt % 4, :]
                else:
                    wslice = w_sb[:, dt, :]
                for j in range(2):
                    su = s0 + j
                    nc.tensor.matmul(
                        prs[j],
                        xt[dt // 4][:, dt % 4, su * 128:(su + 1) * 128],
                        wslice,
                        start=(dt == 0), stop=(dt == DT - 1))
            for j in range(2):
                su = s0 + j
                if kind == "v":
                    nc.vector.tensor_copy(
                        v_sb[:, st * 4 + su, :, 0:64],
                        prs[j].rearrange("p (h d) -> p h d", d=DH))
                else:
                    norm_rope(prs[j], qTall if kind == "q" else kTall,
                              st, su)

        def proj_all(st):
            for w_sb, kind in ((wq_sb, "q"), (wk_sb, "k"), (wv_sb, "v")):
                for s0 in (0, 2):
                    proj_half(w_sb, kind, st, s0)

        def attn_pair(i, ha, hb):
            """Head-paired attention: heads (h, h+2) share PE tile config
            (same hp), so lg and pv matmuls run in same-config groups of 4
            with alternating PSUM banks."""
            last = 4 * i + 3
            npr = 2 * (i + 1)
            if True:
                hp = (ha % 2) * 64
                ets = {ha: ha // 2, hb: hb // 2}
                pvs = {h: psA.tile([128, 512], f32, tag="ps",
                                   name=f"pv{i}_{h}")
                       for h in (ha, hb)}
                lgs = {}

                def lg4(p):
                    for h in (ha, hb):
                        lgs[(h, p)] = psL.tile(
                            [128, 2, 512], f32, tag="lg",
                            name=f"lg{i}_{h}_{p}")
                    for b in range(2):
                        sjb = 2 * p + b
                        r = sjb - 4 * i
                        c0 = r * 128 if r > 0 else 0
                        for h in (ha, hb):
                            nc.tensor.matmul(
                                lgs[(h, p)][:, b, c0:],
                                kTall[hp:hp + 64, ets[h],
                                      sjb * 128:(sjb + 1) * 128],
                                qTall[hp:hp + 64, ets[h],
                                      i * 512 + c0:(i + 1) * 512],
                                start=True, stop=True)

                lg4(0)
                for p in range(npr):
                    exs = {}
                    for h in (ha, hb):
                        lg2 = lgs.pop((h, p))
                        ex = expool.tile([128, 2, 512], bf16, tag="ex",
                                         name=f"ex{i}_{h}_{p}")
                        if 2 * p - 4 * i >= 0:  # diagonal pair: match trim
                            for b in range(2):
                                c0 = max(0, (2 * p + b - 4 * i)) * 128
                                nc.scalar.activation(ex[:, b, c0:],
                                                     lg2[:, b, c0:], AF.Exp)
                        else:
                            nc.scalar.activation(ex, lg2, AF.Exp)
                        exs[h] = ex
                    if p + 1 < npr:
                        lg4(p + 1)
                    for b in range(2):
                        sjb = 2 * p + b
                        r = sjb - 4 * i
                        c0 = r * 128 if r > 0 else 0
                        if r >= 0:
                            for h in (ha, hb):
                                nc.gpsimd.tensor_mul(
                                    exs[h][:, b, r * 128:(r + 1) * 128],
                                    exs[h][:, b, r * 128:(r + 1) * 128],
                                    tri)
                        for h in (ha, hb):
                            nc.tensor.matmul(
                                pvs[h][0:66, c0:],
                                v_sb[:, sjb, h, :],
                                exs[h][:, b, c0:],
                                start=(sjb == 0), stop=(sjb == last))
                for h in (ha, hb):
                    nc.vector.tensor_copy(stash[:, h, :], pvs[h][0:65, :])

        def normalize_gather(i):
            den = work.tile([8, 512], bf16, tag="den", bufs=2)
            nc.scalar.dma_start(den, stash[64:65, :, :])
            return den

        def normalize_recip(i, den):
            """den rows -> si-partition layout via PE transposes -> one
            cheap [128,32] DVE reciprocal."""
            invT = psA.tile([128, 32], bf16, tag="ps")
            for c in range(4):
                nc.tensor.transpose(
                    invT[:, c * 8:(c + 1) * 8],
                    den[:, c * 128:(c + 1) * 128], ident[0:8, 0:8])
            inv_sb = work.tile([128, 32], f32, tag="invsb", bufs=2)
            nc.vector.reciprocal(inv_sb, invT)
            return inv_sb

        def normalize_apply(i, inv_sb):
            """transpose back to row layout, rank-8 indicator broadcast,
            per-head mul into qTall."""
            invrow = psA.tile([8, 4, 128], f32, tag="ps")
            for c in range(4):
                nc.tensor.transpose(
                    invrow[:, c, :], inv_sb[:, c * 8:(c + 1) * 8], identf)
            inv_row = work.tile([8, 512], bf16, tag="invrowsb", bufs=2)
            nc.vector.tensor_copy(
                inv_row, invrow.rearrange("p c j -> p (c j)"))
            for h in range(HPC):
                et, hp = h // 2, (h % 2) * 64
                bc = psA.tile([64, 512], f32, tag="ps", name=f"bc{i}_{h}")
                nc.tensor.matmul(bc, ind8[:, h * 64:(h + 1) * 64], inv_row,
                                 start=True, stop=True)
                nc.vector.tensor_mul(
                    qTall[hp:hp + 64, et, i * 512:(i + 1) * 512],
                    stash[0:64, h, :], bc)

        def yproj_quarter(ib):
            if True:
                for nd0 in (0, 2):
                    pss = [psA.tile([128, 512], f32, tag="ps",
                                    name=f"y{ib}_{nd0 + j}")
                           for j in range(2)]
                    for ket in range(ET):
                        for j in range(2):
                            nd = nd0 + j
                            nc.tensor.matmul(
                                pss[j],
                                qTall[:, ket, ib * 128:(ib + 1) * 128],
                                wo_sb[:, ket, nd * 512:(nd + 1) * 512],
                                start=(ket == 0), stop=(ket == ET - 1))
                    for j in range(2):
                        nd = nd0 + j
                        ys = work.tile([128, 512], f32, tag="ys", bufs=2)
                        if nd % 2 == 0:
                            nc.vector.tensor_copy(ys, pss[j])
                        else:
                            nc.scalar.copy(ys, pss[j])
                        nc.sync.dma_start(
                            Y[ib * 128:(ib + 1) * 128,
                              nd * 512:(nd + 1) * 512], ys)

        def yproj_block(i):
            for ib in range(4 * i, 4 * i + 4):
                yproj_quarter(ib)

        xtiles[1] = load_x(1)
        proj_all(0)
        pairs = ((0, 2), (4, 6), (1, 3), (5, 7))
        halves = ((wq_sb, "q", 0), (wq_sb, "q", 2), (wk_sb, "k", 0),
                  (wk_sb, "k", 2), (wv_sb, "v", 0), (wv_sb, "v", 2))
        for st in range(SB):
            nxt = st + 1 < SB
            if st + 2 < SB:
                xtiles[st + 2] = load_x(st + 2)
            fillers = []
            if nxt:
                fillers += [(lambda w=w, k=k, s=s: proj_half(w, k, st + 1, s))
                            for (w, k, s) in halves]
            if st > 0:
                fillers += [(lambda b=ib: yproj_quarter(b))
                            for ib in range(4 * (st - 1), 4 * st)]
            fi = 0
            per_pair = (len(fillers) + 5) // 6  # spread over 4 pairs + 2 slots
            for idx, (ha, hb) in enumerate(pairs):
                attn_pair(st, ha, hb)
                for _ in range(per_pair):
                    if fi < len(fillers):
                        fillers[fi]()
                        fi += 1
            den = normalize_gather(st)
            inv_sb = normalize_recip(st, den)
            if fi < len(fillers):
                fillers[fi]()
                fi += 1
            normalize_apply(st, inv_sb)
            while fi < len(fillers):
                fillers[fi]()
                fi += 1
        yproj_block(SB - 1)

    return nc


def _host_prep(x, wq, wk, wv, wo, qk_scale):
    """Returns per-core input dicts."""
    perm = np.concatenate([np.arange(0, DH, 2), np.arange(1, DH, 2)])
    wq_n = _l2n(wq, -1).reshape(HEADS, DH, DIM)[:, perm, :].reshape(HEADS * DH, DIM)
    wk_n = _l2n(wk, -1).reshape(HEADS, DH, DIM)[:, perm, :].reshape(HEADS * DH, DIM)
    wv_n = _l2n(wv, -1)
    wo_n = _l2n(wo, 0)
    sp = qk_scale.astype(np.float64)[perm]

    # rope tables with qk_scale folded in; permuted-block layout
    half = np.arange(0, DH, 2)
    freqs = 1.0 / (THETA ** (half.astype(np.float64) / DH))      # (32,)
    ang = np.arange(S, dtype=np.float64)[:, None] * freqs[None]  # (S, 32)
    cos_h, sin_h = np.cos(ang), np.sin(ang)
    cos_p = np.concatenate([cos_h, cos_h], 1)                    # (S, 64)
    sin_e = np.concatenate([-sin_h, sin_h], 1)
    cos_eff = (cos_p * sp[None, :]).astype(np.float32)
    swap_sp = np.concatenate([sp[32:], sp[:32]])
    sin_eff = (sin_e * swap_sp[None, :]).astype(np.float32)
    # device layout [128, SS*DH]: [p, b*64+c] = tbl[b*128+p, c]
    cosd = np.ascontiguousarray(
        cos_eff.reshape(SS, 128, DH).transpose(1, 0, 2).reshape(128, SS * DH))
    sind = np.ascontiguousarray(
        sin_eff.reshape(SS, 128, DH).transpose(1, 0, 2).reshape(128, SS * DH))

    # causal triangle for the diagonal 128-blocks: keep sjl <= sil
    sjl = np.arange(128)[:, None]
    sil = np.arange(128)[None, :]
    trid = (sjl <= sil).astype(np.float32)

    # indicator for denominator broadcast: ind8[k, h*64+m] = (k == h)
    ind8 = np.zeros((8, 512), dtype=np.float32)
    for h in range(8):
        ind8[h, h * 64:(h + 1) * 64] = 1.0

    in_maps = []
    for c in range(NCORES):
        b, t = divmod(c, TP)
        e0 = t * E
        in_maps.append({
            "xT": np.ascontiguousarray(x[b].T).astype(BF16),
            "wqT": np.ascontiguousarray(wq_n[e0:e0 + E].T).astype(BF16),
            "wkT": np.ascontiguousarray(wk_n[e0:e0 + E].T).astype(BF16),
            "wvT": np.ascontiguousarray(wv_n[e0:e0 + E].T).astype(BF16),
            "woT": np.ascontiguousarray(wo_n[:, e0:e0 + E].T).astype(BF16),
            "cosd": cosd.astype(BF16), "sind": sind.astype(BF16),
            "trid": trid.astype(BF16), "ind8d": ind8.astype(BF16),
        })
    return in_maps


def _install_profile_hook():
    """antenv.axon_hooks is absent in this image; shim it and register the
    ctypes NTFF hook against /opt/axon/libaxon_pjrt.so (mirrors trn_boot)."""
    import types
    import ctypes
    import contextlib

    try:
        from antenv.axon_hooks import get_axon_ntff_profile_hook  # noqa
        return
    except ImportError:
        pass
    import antenv
    mod = types.ModuleType("antenv.axon_hooks")
    state = {}
    mod.set_axon_ntff_profile_hook = lambda h: state.__setitem__("h", h)
    mod.get_axon_ntff_profile_hook = lambda: state.get("h")
    sys.modules["antenv.axon_hooks"] = mod
    antenv.axon_hooks = mod

    so_path = "/opt/axon/libaxon_pjrt.so"
    lib = ctypes.CDLL(so_path)
    if not hasattr(lib, "axon_start_nrt_profile"):
        return
    lib.axon_start_nrt_profile.argtypes = [
        ctypes.POINTER(ctypes.c_int64), ctypes.c_size_t]
    lib.axon_start_nrt_profile.restype = ctypes.c_int64
    lib.axon_stop_nrt_profile.argtypes = [ctypes.c_char_p]
    lib.axon_stop_nrt_profile.restype = ctypes.c_int64

    @contextlib.contextmanager
    def _hook(output_dir, device_ids):
        import jax
        jax.devices()
        if device_ids:
            ids = (ctypes.c_int64 * len(device_ids))(*device_ids)
            rc = lib.axon_start_nrt_profile(ids, len(device_ids))
        else:
            rc = lib.axon_start_nrt_profile(None, 0)
        if rc != 0:
            raise RuntimeError(f"axon_start_nrt_profile rc={rc}")
        try:
            yield
        finally:
            n = lib.axon_stop_nrt_profile(str(output_dir).encode())
            print(f"profile: {n} file(s) written to {output_dir}",
                  file=sys.stderr)

    mod.set_axon_ntff_profile_hook(_hook)


def kernel(x, wq, wk, wv, wo, qk_scale, _profile=False):
    from concourse.bass_utils import run_bass_kernel_spmd

    if _profile:
        _install_profile_hook()

    if "nc" not in _CACHE:
        nc = _build_program()
        nc.finalize()
        _CACHE["nc"] = nc
    nc = _CACHE["nc"]
    in_maps = _host_prep(np.asarray(x), np.asarray(wq), np.asarray(wk),
                         np.asarray(wv), np.asarray(wo), np.asarray(qk_scale))
    res = run_bass_kernel_spmd(nc, in_maps, core_ids=list(range(NCORES)),
                               trace=_profile)
    outs = res.results
    y = np.empty((B, S, DIM), dtype=np.float32)
    for b in range(B):
        y[b] = sum(outs[b * TP + t]["Y"] for t in range(TP))
    if _profile:
        _CACHE["last_exec_time_ns"] = res.exec_time_ns
        _CACHE["last_profile"] = res.profile_json
    return y


# revision 23
# speedup vs baseline: 1.3790x; 1.0183x over previous
"""nn_Attention Trainium2 Bass kernel (v2 — interleaved pipeline).

Full attention forward: x->(q,k,v) with l2-normalized weights, per-head-dim
l2 norm + learned qk scale, interleaved RoPE, causal SDPA, output projection
with column-l2-normalized wo.

Sharding: TP=4 over heads (8 heads/core) x DP=2 over batch across 8 cores.
Each core computes a partial [2048, 2048] output for its batch; host sums
the 4 TP partials per batch.

v2 changes vs v1:
- single interleaved loop per 512-row block: proj -> attention -> yproj,
  so DVE rope work, Act exp work and PE matmuls overlap across phases.
- q/k transposes via DMA xbar (dma_start_transpose) instead of PE
  transposes + DVE copies.
- causal mask as a single 128x128 triangle multiply on the Pool engine.
- lg/pv matmuls trimmed to the live columns on diagonal blocks.
- softmax denominators: v's 65th ones-column -> psum row 64 -> stashed ->
  gathered by DMA -> PE-transposed to si-partition layout -> one cheap
  [128,32] reciprocal -> transposed back -> rank-8 indicator matmul
  broadcast (replaces 3.3us-per-call wide DVE reciprocals).
- x streamed per 512-column block (2-deep) instead of fully resident.
- yproj results DMA'd directly from PSUM to DRAM.
"""
import sys
import os
import math
from contextlib import ExitStack

sys.path.insert(0, "/opt/trn_rl_repo")

import numpy as np
import ml_dtypes

BF16 = ml_dtypes.bfloat16

B, S, DIM = 2, 2048, 2048
HEADS, DH = 32, 64
THETA = 10000.0
NCORES = 8
TP = 4             # head-parallel ways
HPC = HEADS // TP  # heads per core = 8
E = HPC * DH       # per-core qkv width = 512
ET = E // 128      # e-tiles per core = 4
DT = DIM // 128    # contraction d-tiles = 16
SB = S // 512      # 512-wide seq blocks = 4
SS = S // 128      # 128-wide seq blocks = 16

_CACHE = {}


def _l2n(w, axis):
    n = np.sqrt((w.astype(np.float64) ** 2).sum(axis=axis, keepdims=True))
    n = np.maximum(n, 1e-12)
    return (w / n).astype(np.float32)


def _build_program():
    import concourse.bass as bass
    from concourse import bacc
    import concourse.mybir as mybir
    import concourse.tile as tile
    from concourse.masks import make_identity

    f32 = mybir.dt.float32
    bf16 = mybir.dt.bfloat16
    AF = mybir.ActivationFunctionType
    AX = mybir.AxisListType
    OP = mybir.AluOpType

    nc = bacc.Bacc("TRN2", target_bir_lowering=False)

    xT = nc.dram_tensor("xT", [DIM, S], bf16, kind="ExternalInput")
    wqT = nc.dram_tensor("wqT", [DIM, E], bf16, kind="ExternalInput")
    wkT = nc.dram_tensor("wkT", [DIM, E], bf16, kind="ExternalInput")
    wvT = nc.dram_tensor("wvT", [DIM, E], bf16, kind="ExternalInput")
    woT = nc.dram_tensor("woT", [E, DIM], bf16, kind="ExternalInput")
    cosd = nc.dram_tensor("cosd", [128, SS * DH], bf16, kind="ExternalInput")
    sind = nc.dram_tensor("sind", [128, SS * DH], bf16, kind="ExternalInput")
    trid = nc.dram_tensor("trid", [128, 128], bf16, kind="ExternalInput")
    ind8d = nc.dram_tensor("ind8d", [8, 512], bf16, kind="ExternalInput")
    Y = nc.dram_tensor("Y", [S, DIM], f32, kind="ExternalOutput")

    with tile.TileContext(nc) as tc, ExitStack() as ctx:
        const = ctx.enter_context(tc.tile_pool(name="const", bufs=1))
        wpool = ctx.enter_context(tc.tile_pool(name="wpool", bufs=4))
        xpool = ctx.enter_context(tc.tile_pool(name="xpool", bufs=2))
        qkv = ctx.enter_context(tc.tile_pool(name="qkv", bufs=1))
        work = ctx.enter_context(tc.tile_pool(name="work", bufs=1))
        expool = ctx.enter_context(tc.tile_pool(name="expool", bufs=4))
        psA = ctx.enter_context(
            tc.tile_pool(name="psA", bufs=4, space="PSUM"))
        psL = ctx.enter_context(
            tc.tile_pool(name="psL", bufs=2, space="PSUM"))

        # --- weights (wq first, quartered, so proj can start early) ---
        wq_sb = [wpool.tile([128, 4, E], bf16, tag=f"wq{j}", bufs=1, name=f"wq{j}")
                 for j in range(4)]
        wk_sb = wpool.tile([128, DT, E], bf16, tag="wk", bufs=1)
        wv_sb = wpool.tile([128, DT, E], bf16, tag="wv", bufs=1)
        wo_sb = wpool.tile([128, ET, DIM], bf16, tag="wo", bufs=1)
        wqr = wqT.rearrange("(t p) e -> p t e", p=128)

        xtiles = {}

        def load_x(st):
            ts = [xpool.tile([128, 4, 512], bf16, tag=f"x{j}", bufs=2,
                             name=f"xst{st}_{j}") for j in range(4)]
            src = xT[:, st * 512:(st + 1) * 512].rearrange(
                "(t p) s -> p t s", p=128)
            for j in range(4):
                nc.sync.dma_start(ts[j], src[:, j * 4:(j + 1) * 4, :])
            return ts

        # interleave wq quarters with x quarters so dt=0..3 can start early
        x0src = xT[:, 0:512].rearrange("(t p) s -> p t s", p=128)
        x0 = [xpool.tile([128, 4, 512], bf16, tag=f"x{j}", bufs=2,
                         name=f"xst0_{j}") for j in range(4)]
        for j in range(4):
            nc.sync.dma_start(wq_sb[j], wqr[:, j * 4:(j + 1) * 4, :])
            nc.sync.dma_start(x0[j], x0src[:, j * 4:(j + 1) * 4, :])
        xtiles[0] = x0
        nc.sync.dma_start(wk_sb, wkT.rearrange("(t p) e -> p t e", p=128))
        nc.sync.dma_start(wv_sb, wvT.rearrange("(t p) e -> p t e", p=128))

        # --- constants ---
        cos_sb = const.tile([128, SS, DH], bf16)
        sin_sb = const.tile([128, SS, DH], bf16)
        nc.sync.dma_start(cos_sb, cosd.rearrange("p (b d) -> p b d", d=DH))
        nc.sync.dma_start(sin_sb, sind.rearrange("p (b d) -> p b d", d=DH))
        tri = const.tile([128, 128], bf16)
        nc.sync.dma_start(tri, trid[:, :])
        ind8 = const.tile([8, 512], bf16)
        nc.sync.dma_start(ind8, ind8d[:, :])
        nc.sync.dma_start(wo_sb, woT.rearrange("(t p) e -> p t e", p=128))
        identf = const.tile([128, 128], f32)
        make_identity(nc, identf)
        ident = const.tile([128, 128], bf16)
        make_identity(nc, ident)

        # --- persistent activations ---
        qTall = qkv.tile([128, ET, S], bf16, tag="qT")
        kTall = qkv.tile([128, ET, S], bf16, tag="kT")
        v_sb = qkv.tile([128, SS, HPC, 66], bf16, tag="v")
        stash = qkv.tile([65, HPC, 512], bf16, tag="stash")
        nc.vector.memset(v_sb[:, :, :, 64:66], 1.0)

        def norm_rope(ps, dstT, st, su):
            """psum [si,e] natural -> per-head l2norm, rope, bf16,
            -> DMA-transpose into dstT columns."""
            sblk = st * 4 + su
            sq = work.tile([128, E], bf16, tag="sq", bufs=2)
            nc.scalar.square(sq, ps)
            ssq = work.tile([128, HPC], f32, tag="ssq", bufs=2)
            nc.vector.tensor_reduce(
                ssq, sq.rearrange("p (h d) -> p h d", d=DH),
                axis=AX.X, op=OP.add)
            # rsqrt via magic-number seed + 2 Newton iterations (DVE only;
            # keeps the Act engine free of sqrt so its activation table
            # never leaves the exp set)
            inv = work.tile([128, HPC], f32, tag="inv", bufs=2)
            ssq_i = ssq.bitcast(mybir.dt.int32)
            inv_i = inv.bitcast(mybir.dt.int32)
            nc.vector.tensor_scalar(inv_i, ssq_i, 1, None,
                                    op0=OP.arith_shift_right)
            nc.vector.tensor_scalar(inv_i, inv_i, 0x5f3759df, -1,
                                    op0=OP.subtract, op1=OP.mult)
            y2 = work.tile([128, HPC], f32, tag="y2", bufs=2)
            for _ in range(2):
                nc.vector.tensor_mul(y2, inv, inv)
                nc.vector.scalar_tensor_tensor(
                    y2, ssq, -0.5, y2, op0=OP.mult, op1=OP.mult)
                nc.vector.tensor_scalar(y2, y2, 1.5, None, op0=OP.add)
                nc.vector.tensor_mul(inv, inv, y2)
            qn = work.tile([128, HPC, DH], bf16, tag="qn", bufs=2)
            nc.vector.tensor_mul(
                qn, ps.rearrange("p (h d) -> p h d", d=DH),
                inv.unsqueeze(2).broadcast_to([128, HPC, DH]))
            cosb = cos_sb[:, sblk:sblk + 1, :].broadcast_to([128, HPC, DH])
            sinb = sin_sb[:, sblk:sblk + 1, :].broadcast_to([128, HPC, DH])
            rot = work.tile([128, HPC, 2, 32], bf16, tag="rot", bufs=2)
            qn4 = qn.rearrange("p h (t u) -> p h t u", u=32)
            nc.vector.tensor_copy(rot[:, :, 0:1, :], qn4[:, :, 1:2, :])
            nc.vector.tensor_copy(rot[:, :, 1:2, :], qn4[:, :, 0:1, :])
            nc.vector.tensor_mul(rot.rearrange("p h t u -> p h (t u)"),
                                 rot.rearrange("p h t u -> p h (t u)"), sinb)
            nc.vector.tensor_mul(qn, qn, cosb)
            qo = work.tile([128, E], bf16, tag="qo", bufs=2)
            nc.vector.tensor_add(
                qo, qn.rearrange("p h d -> p (h d)"),
                rot.rearrange("p h t u -> p (h t u)"))
            nc.sync.dma_start_transpose(
                dstT[:, :, sblk * 128:(sblk + 1) * 128], qo)

        def proj_half(w_sb, kind, st, s0):
            xt = xtiles[st]
            quartered = isinstance(w_sb, list)
            prs = [psA.tile([128, E], f32, tag="ps",
                            name=f"p{kind}{st}_{s0 + j}")
                   for j in range(2)]
            for dt in range(DT):
                if quartered:
                    wslice = w_sb[dt // 4][:, dt % 4, :]
                else:
                    wslice = w_sb[:, dt, :]
                for j in range(2):
                    su = s0 + j
                    nc.tensor.matmul(
                        prs[j],
                        xt[dt // 4][:, dt % 4, su * 128:(su + 1) * 128],
                        wslice,
                        start=(dt == 0), stop=(dt == DT - 1))
            for j in range(2):
                su = s0 + j
                if kind == "v":
                    nc.vector.tensor_copy(
                        v_sb[:, st * 4 + su, :, 0:64],
                        prs[j].rearrange("p (h d) -> p h d", d=DH))
                else:
                    norm_rope(prs[j], qTall if kind == "q" else kTall,
                              st, su)

        def proj_all(st):
            for w_sb, kind in ((wq_sb, "q"), (wk_sb, "k"), (wv_sb, "v")):
                for s0 in (0, 2):
                    proj_half(w_sb, kind, st, s0)

        def attn_pair(i, ha, hb):
            """Head-paired attention: heads (h, h+2) share PE tile config
            (same hp), so lg and pv matmuls run in same-config groups of 4
            with alternating PSUM banks."""
            last = 4 * i + 3
            npr = 2 * (i + 1)
            if True:
                hp = (ha % 2) * 64
                ets = {ha: ha // 2, hb: hb // 2}
                pvs = {h: psA.tile([128, 512], f32, tag="ps",
                                   name=f"pv{i}_{h}")
                       for h in (ha, hb)}
                lgs = {}

                def lg4(p):
                    for h in (ha, hb):
                        lgs[(h, p)] = psL.tile(
                            [128, 2, 512], f32, tag="lg",
                            name=f"lg{i}_{h}_{p}")
                    for b in range(2):
                        sjb = 2 * p + b
                        r = sjb - 4 * i
                        c0 = r * 128 if r > 0 else 0
                        for h in (ha, hb):
                            nc.tensor.matmul(
                                lgs[(h, p)][:, b, c0:],
                                kTall[hp:hp + 64, ets[h],
                                      sjb * 128:(sjb + 1) * 128],
                                qTall[hp:hp + 64, ets[h],
                                      i * 512 + c0:(i + 1) * 512],
                                start=True, stop=True)

                lg4(0)
                for p in range(npr):
                    exs = {}
                    for h in (ha, hb):
                        lg2 = lgs.pop((h, p))
                        ex = expool.tile([128, 2, 512], bf16, tag="ex",
                                         name=f"ex{i}_{h}_{p}")
                        if 2 * p - 4 * i >= 0:  # diagonal pair: match trim
                            for b in range(2):
                                c0 = max(0, (2 * p + b - 4 * i)) * 128
                                nc.scalar.activation(ex[:, b, c0:],
                                                     lg2[:, b, c0:], AF.Exp)
                        else:
                            nc.scalar.activation(ex, lg2, AF.Exp)
                        exs[h] = ex
                    if p + 1 < npr:
                        lg4(p + 1)
                    for b in range(2):
                        sjb = 2 * p + b
                        r = sjb - 4 * i
                        c0 = r * 128 if r > 0 else 0
                        if r >= 0:
                            for h in (ha, hb):
                                nc.gpsimd.tensor_mul(
                                    exs[h][:, b, r * 128:(r + 1) * 128],
                                    exs[h][:, b, r * 128:(r + 1) * 128],
                                    tri)
                        for h in (ha, hb):
                            nc.tensor.matmul(
                                pvs[h][0:66, c0:],
                                v_sb[:, sjb, h, :],
                                exs[h][:, b, c0:],
                                start=(sjb == 0), stop=(sjb == last))
                for h in (ha, hb):
                    nc.vector.tensor_copy(stash[:, h, :], pvs[h][0:65, :])

        def normalize_gather(i):
            den = work.tile([8, 512], bf16, tag="den", bufs=2)
            nc.scalar.dma_start(den, stash[64:65, :, :])
            return den

        def normalize_recip(i, den):
            """den rows -> si-partition layout via PE transposes -> one
            cheap [128,32] DVE reciprocal."""
            invT = psA.tile([128, 32], bf16, tag="ps")
            for c in range(4):
                nc.tensor.transpose(
                    invT[:, c * 8:(c + 1) * 8],
                    den[:, c * 128:(c + 1) * 128], ident[0:8, 0:8])
            inv_sb = work.tile([128, 32], f32, tag="invsb", bufs=2)
            nc.vector.reciprocal(inv_sb, invT)
            return inv_sb

        def normalize_apply(i, inv_sb):
            """transpose back to row layout, rank-8 indicator broadcast,
            per-head mul into qTall."""
            invrow = psA.tile([8, 4, 128], f32, tag="ps")
            for c in range(4):
                nc.tensor.transpose(
                    invrow[:, c, :], inv_sb[:, c * 8:(c + 1) * 8], identf)
            inv_row = work.tile([8, 512], bf16, tag="invrowsb", bufs=2)
            nc.vector.tensor_copy(
                inv_row, invrow.rearrange("p c j -> p (c j)"))
            for h in range(HPC):
                et, hp = h // 2, (h % 2) * 64
                bc = psA.tile([64, 512], f32, tag="ps", name=f"bc{i}_{h}")
                nc.tensor.matmul(bc, ind8[:, h * 64:(h + 1) * 64], inv_row,
                                 start=True, stop=True)
                nc.vector.tensor_mul(
                    qTall[hp:hp + 64, et, i * 512:(i + 1) * 512],
                    stash[0:64, h, :], bc)

        def yproj_quarter(ib):
            if True:
                for nd0 in (0, 2):
                    pss = [psA.tile([128, 512], f32, tag="ps",
                                    name=f"y{ib}_{nd0 + j}")
                           for j in range(2)]
                    for ket in range(ET):
                        for j in range(2):
                            nd = nd0 + j
                            nc.tensor.matmul(
                                pss[j],
                                qTall[:, ket, ib * 128:(ib + 1) * 128],
                                wo_sb[:, ket, nd * 512:(nd + 1) * 512],
                                start=(ket == 0), stop=(ket == ET - 1))
                    for j in range(2):
                        nd = nd0 + j
                        ys = work.tile([128, 512], f32, tag="ys", bufs=2)
                        if nd % 2 == 0:
                            nc.vector.tensor_copy(ys, pss[j])
                        else:
                            nc.scalar.copy(ys, pss[j])
                        nc.sync.dma_start(
                            Y[ib * 128:(ib + 1) * 128,
                              nd * 512:(nd + 1) * 512], ys)

        def yproj_block(i):
            for ib in range(4 * i, 4 * i + 4):
                yproj_quarter(ib)

        xtiles[1] = load_x(1)
        proj_all(0)
        yq_backlog = []
        pairs = ((0, 2), (4, 6), (1, 3), (5, 7))
        halves = ((wq_sb, "q", 0), (wq_sb, "q", 2), (wk_sb, "k", 0),
                  (wk_sb, "k", 2), (wv_sb, "v", 0), (wv_sb, "v", 2))
        for st in range(SB):
            nxt = st + 1 < SB
            if st + 2 < SB:
                xtiles[st + 2] = load_x(st + 2)
            fillers = []
            if nxt:
                fillers += [(lambda w=w, k=k, s=s: proj_half(w, k, st + 1, s))
                            for (w, k, s) in halves]
            if st > 0:
                yq_backlog.extend(range(4 * (st - 1), 4 * st))
            ntake = 2 if nxt else len(yq_backlog)
            for _ in range(min(ntake, len(yq_backlog))):
                ib = yq_backlog.pop(0)
                fillers.append(lambda b=ib: yproj_quarter(b))
            fi = 0
            per_pair = (len(fillers) + 5) // 6  # spread over 4 pairs + 2 slots
            for idx, (ha, hb) in enumerate(pairs):
                attn_pair(st, ha, hb)
                for _ in range(per_pair):
                    if fi < len(fillers):
                        fillers[fi]()
                        fi += 1
            den = normalize_gather(st)
            inv_sb = normalize_recip(st, den)
            if fi < len(fillers):
                fillers[fi]()
                fi += 1
            normalize_apply(st, inv_sb)
            while fi < len(fillers):
                fillers[fi]()
                fi += 1
        yproj_block(SB - 1)

    return nc


def _host_prep(x, wq, wk, wv, wo, qk_scale):
    """Returns per-core input dicts."""
    perm = np.concatenate([np.arange(0, DH, 2), np.arange(1, DH, 2)])
    wq_n = _l2n(wq, -1).reshape(HEADS, DH, DIM)[:, perm, :].reshape(HEADS * DH, DIM)
    wk_n = _l2n(wk, -1).reshape(HEADS, DH, DIM)[:, perm, :].reshape(HEADS * DH, DIM)
    wv_n = _l2n(wv, -1)
    wo_n = _l2n(wo, 0)
    sp = qk_scale.astype(np.float64)[perm]

    # rope tables with qk_scale folded in; permuted-block layout
    half = np.arange(0, DH, 2)
    freqs = 1.0 / (THETA ** (half.astype(np.float64) / DH))      # (32,)
    ang = np.arange(S, dtype=np.float64)[:, None] * freqs[None]  # (S, 32)
    cos_h, sin_h = np.cos(ang), np.sin(ang)
    cos_p = np.concatenate([cos_h, cos_h], 1)                    # (S, 64)
    sin_e = np.concatenate([-sin_h, sin_h], 1)
    cos_eff = (cos_p * sp[None, :]).astype(np.float32)
    swap_sp = np.concatenate([sp[32:], sp[:32]])
    sin_eff = (sin_e * swap_sp[None, :]).astype(np.float32)
    # device layout [128, SS*DH]: [p, b*64+c] = tbl[b*128+p, c]
    cosd = np.ascontiguousarray(
        cos_eff.reshape(SS, 128, DH).transpose(1, 0, 2).reshape(128, SS * DH))
    sind = np.ascontiguousarray(
        sin_eff.reshape(SS, 128, DH).transpose(1, 0, 2).reshape(128, SS * DH))

    # causal triangle for the diagonal 128-blocks: keep sjl <= sil
    sjl = np.arange(128)[:, None]
    sil = np.arange(128)[None, :]
    trid = (sjl <= sil).astype(np.float32)

    # indicator for denominator broadcast: ind8[k, h*64+m] = (k == h)
    ind8 = np.zeros((8, 512), dtype=np.float32)
    for h in range(8):
        ind8[h, h * 64:(h + 1) * 64] = 1.0

    in_maps = []
    for c in range(NCORES):
        b, t = divmod(c, TP)
        e0 = t * E
        in_maps.append({
            "xT": np.ascontiguousarray(x[b].T).astype(BF16),
            "wqT": np.ascontiguousarray(wq_n[e0:e0 + E].T).astype(BF16),
            "wkT": np.ascontiguousarray(wk_n[e0:e0 + E].T).astype(BF16),
            "wvT": np.ascontiguousarray(wv_n[e0:e0 + E].T).astype(BF16),
            "woT": np.ascontiguousarray(wo_n[:, e0:e0 + E].T).astype(BF16),
            "cosd": cosd.astype(BF16), "sind": sind.astype(BF16),
            "trid": trid.astype(BF16), "ind8d": ind8.astype(BF16),
        })
    return in_maps


def _install_profile_hook():
    """antenv.axon_hooks is absent in this image; shim it and register the
    ctypes NTFF hook against /opt/axon/libaxon_pjrt.so (mirrors trn_boot)."""
    import types
    import ctypes
    import contextlib

    try:
        from antenv.axon_hooks import get_axon_ntff_profile_hook  # noqa
        return
    except ImportError:
        pass
    import antenv
    mod = types.ModuleType("antenv.axon_hooks")
    state = {}
    mod.set_axon_ntff_profile_hook = lambda h: state.__setitem__("h", h)
    mod.get_axon_ntff_profile_hook = lambda: state.get("h")
    sys.modules["antenv.axon_hooks"] = mod
    antenv.axon_hooks = mod

    so_path = "/opt/axon/libaxon_pjrt.so"
    lib = ctypes.CDLL(so_path)
    if not hasattr(lib, "axon_start_nrt_profile"):
        return
    lib.axon_start_nrt_profile.argtypes = [
        ctypes.POINTER(ctypes.c_int64), ctypes.c_size_t]
    lib.axon_start_nrt_profile.restype = ctypes.c_int64
    lib.axon_stop_nrt_profile.argtypes = [ctypes.c_char_p]
    lib.axon_stop_nrt_profile.restype = ctypes.c_int64

    @contextlib.contextmanager
    def _hook(output_dir, device_ids):
        import jax
        jax.devices()
        if device_ids:
            ids = (ctypes.c_int64 * len(device_ids))(*device_ids)
            rc = lib.axon_start_nrt_profile(ids, len(device_ids))
        else:
            rc = lib.axon_start_nrt_profile(None, 0)
        if rc != 0:
            raise RuntimeError(f"axon_start_nrt_profile rc={rc}")
        try:
            yield
        finally:
            n = lib.axon_stop_nrt_profile(str(output_dir).encode())
            print(f"profile: {n} file(s) written to {output_dir}",
                  file=sys.stderr)

    mod.set_axon_ntff_profile_hook(_hook)


def kernel(x, wq, wk, wv, wo, qk_scale, _profile=False):
    from concourse.bass_utils import run_bass_kernel_spmd

    if _profile:
        _install_profile_hook()

    if "nc" not in _CACHE:
        nc = _build_program()
        nc.finalize()
        _CACHE["nc"] = nc
    nc = _CACHE["nc"]
    in_maps = _host_prep(np.asarray(x), np.asarray(wq), np.asarray(wk),
                         np.asarray(wv), np.asarray(wo), np.asarray(qk_scale))
    res = run_bass_kernel_spmd(nc, in_maps, core_ids=list(range(NCORES)),
                               trace=_profile)
    outs = res.results
    y = np.empty((B, S, DIM), dtype=np.float32)
    for b in range(B):
        y[b] = sum(outs[b * TP + t]["Y"] for t in range(TP))
    if _profile:
        _CACHE["last_exec_time_ns"] = res.exec_time_ns
        _CACHE["last_profile"] = res.profile_json
    return y
